# revision 16
# baseline (speedup 1.0000x reference)
"""LoRA MultiheadAttention on 8 NeuronCores (Bass/Tile), v6.

Sharding: 32 (batch, head) attention slices -> 4 heads x 1 batch per core.
Cores 0-3 take batch 0, cores 4-7 batch 1; core c handles heads
(c%4)*4 .. (c%4)*4+3, i.e. a contiguous 256-wide slice of the head dims.

The PE drains one 128-row PSUM column per cycle regardless of K/M/dtype
(fp8 DoubleRow only doubles K per instruction; measured, not 2x drain),
so v6 cuts WALL time with PE tile concurrency (measured ~2x on HW):

  scores:  K=64 per head -> row-group pairs. kp2[hp] stacks head 2hp at
           partitions 0:64 and head 2hp+1 at 64:128; the two K=64 matmuls
           (tile_position (0,0)/(64,0)) run concurrently: 2x score rate.
  LoRA A:  M=48 -> column-group pairs at out partitions 0:48 / 64:112.
  bias folds: no ones row in X^T (8 k-tiles, not 9). K/V biases ride the
           LoRA extra matmul (bias row 16 of kb/vb against an all-ones
           row 16 of ak/av); V's softmax-denominator ones column also
           comes from that row. Q bias is added during PSUM->SBUF evac
           (per-partition bias via ACT activation / DVE tensor_scalar).
  exp:     per (head-pair, tj): 4 [128,512] chunks; ACT does the real
           exp of one head, DVE the Schraudolph bf16 exp of the other
           (i16 = rint(s*128/ln2 + B) bitcast bf16), swapping heads
           every chunk for noise spreading. ~50% Schraudolph.
  norm:    denominator row -> [128,8] via DMA, 128-lane reciprocal,
           DRAM reshape, stride-0 broadcast; the even head's oT multiply
           runs on GpSimd (SBUF-only engine, otherwise idle), the odd
           head's on DVE (needs the partition shift).
  filler:  B (Q^T/K^T), C (V row-tiles) and the half-0 out-projection
           drain into the attention PE stream; epilogue does half-1 in
           two 8-chunk waves with the last norm between kt2 passes.
  head/tail: first k-tile's DMAs split across rings and dispatched from
           both Sync and ACT queues; output DMAs partition-split.

b_v/b_k fold into LoRA rows; out_b added on host.
"""

import sys

sys.path.insert(0, "/opt/trn_rl_repo")

import math
from contextlib import ExitStack

import ml_dtypes
import numpy as np

import concourse.bass as bass
import concourse.tile as tile
from concourse import mybir
from concourse.alu_op_type import AluOpType
from concourse.bass_utils import run_bass_kernel_spmd

BF16 = ml_dtypes.bfloat16
F32 = mybir.dt.float32
BF = mybir.dt.bfloat16
I16 = mybir.dt.int16

T = 2048
D = 1024
H = 16
HD = 64
R = 16
BSZ = 2
SCALE = 16.0
NCORES = 8
HPC = 4  # heads per core
CD = HPC * HD  # 256 head dims per core
VW = HD + 1  # V block width per head (ones column appended)
NKT = 8  # 1024 contraction rows = 8 k-tiles (biases folded elsewhere)
P = 128
NTT = T // P  # 16 row tiles
HF = T // 2  # 1024: queries processed in two halves

# Schraudolph-bf16 exp: i16 = rint(x * 128/ln2 + (127*128 - C)), bitcast bf16
EXP_A = 128.0 / math.log(2.0)
EXP_B = 127.0 * 128.0 - 7.3


def build_nc():
    nc = bass.Bass()
    xa = nc.dram_tensor("xa", [NKT, P, T], BF, kind="ExternalInput")
    wqk = nc.dram_tensor("wqk", [NKT, P, 2 * CD], BF, kind="ExternalInput")
    wv = nc.dram_tensor("wv", [NKT, P, HPC * VW], BF, kind="ExternalInput")
    ab = nc.dram_tensor("ab", [NKT, P, 3 * R], BF, kind="ExternalInput")
    qb = nc.dram_tensor("qb", [2, P], F32, kind="ExternalInput")
    kbm = nc.dram_tensor("kbm", [R + 1, CD], BF, kind="ExternalInput")
    vbm = nc.dram_tensor("vbm", [R + 1, HPC * VW], BF, kind="ExternalInput")
    wo = nc.dram_tensor("wo", [2, P, D], BF, kind="ExternalInput")
    out = nc.dram_tensor("out", [NTT, P, D], BF, kind="ExternalOutput")

    with tile.TileContext(nc) as tc, ExitStack() as ctx:
        singles = ctx.enter_context(tc.tile_pool(name="singles", bufs=1))

        xa_t = [singles.tile([P, T], BF, name=f"xa{i}", tag=f"xa{i}") for i in range(NKT)]
        wqk_t = [singles.tile([P, 2 * CD], BF, name=f"wqk{i}", tag=f"wqk{i}") for i in range(NKT)]
        wv_t = [singles.tile([P, HPC * VW], BF, name=f"wv{i}", tag=f"wv{i}") for i in range(NKT)]
        ab_t = [singles.tile([P, 3 * R], BF, name=f"ab{i}", tag=f"ab{i}") for i in range(NKT)]
        qb_t = singles.tile([P, 2], F32, tag="qb")
        kb_t = singles.tile([P, CD], BF, tag="kb")
        vb_t = singles.tile([P, HPC * VW], BF, tag="vb")
        ak_sb = singles.tile([P, T], BF, tag="ak")
        av_sb = singles.tile([P, T], BF, tag="av")
        qk_sb = [singles.tile([P, T], BF, name=f"qk{i}", tag=f"qk{i}") for i in range(2)]
        kp2_sb = [singles.tile([P, T], BF, name=f"kp{i}", tag=f"kp{i}") for i in range(2)]
        v_sb = [singles.tile([P, HPC * VW], BF, name=f"v{i}", tag=f"v{i}") for i in range(NTT)]
        oT_sb = [singles.tile([P, T], BF, name=f"oT{i}", tag=f"oT{i}") for i in range(2)]
        wo_t = [singles.tile([P, D], BF, name=f"wo{i}", tag=f"wo{i}") for i in range(2)]

        # memsets first: DVE is idle while the DMA queues spin up.
        # ak/av zero except the all-ones row at partition 32 that applies
        # the K/V bias row 32 of kb/vb (32-aligned engine partition base).
        nc.gpsimd.memset(ak_sb, 0.0)
        nc.vector.memset(ak_sb[32:33, :], 1.0)
        nc.gpsimd.memset(av_sb, 0.0)
        nc.vector.memset(av_sb[32:33, :], 1.0)
        nc.vector.memset(kb_t, 0.0)
        nc.vector.memset(vb_t, 0.0)
        # dummy exp: pull the ACT exp table load into the DMA-bound head
        nc.scalar.activation(vb_t[0:1, 0:2], kb_t[0:1, 0:2],
                             mybir.ActivationFunctionType.Exp)

        # input loads: first k-tile split across rings, alternating the two
        # HW-DGE dispatch queues (Sync + ACT) so descriptors land ~2x faster
        eng_i = [0]

        def ld(dst, src):
            eng_i[0] += 1
            (nc.sync if eng_i[0] % 2 else nc.scalar).dma_start(out=dst, in_=src)

        for p4 in range(4):
            ps = slice(p4 * 32, (p4 + 1) * 32)
            ld(ab_t[0][ps, :], ab[0, ps, :])
        for c4 in range(4):
            cs = slice(c4 * 512, (c4 + 1) * 512)
            for p2 in range(2):
                ps = slice(p2 * 64, (p2 + 1) * 64)
                ld(xa_t[0][ps, cs], xa[0, ps, cs])
        for p2 in range(2):
            ps = slice(p2 * 64, (p2 + 1) * 64)
            ld(wqk_t[0][ps, :], wqk[0, ps, :])
        # qb arrives transposed [2,128] -> [128,2] via AP swap (tiny but
        # descriptor-heavy; dispatch early, needed by the first Q evac)
        ld(qb_t, qb[:, :].rearrange("a b -> b a"))
        for i in range(1, NKT):
            ld(ab_t[i], ab[i, :, :])
            ld(xa_t[i], xa[i, :, :])
            ld(wqk_t[i], wqk[i, :, :])
        ld(kb_t[0:R, :], kbm[0:R, :])
        ld(kb_t[32:33, :], kbm[R : R + 1, :])
        ld(vb_t[0:R, :], vbm[0:R, :])
        ld(vb_t[32:33, :], vbm[R : R + 1, :])
        for i in range(NKT):
            ld(wv_t[i], wv[i, :, :])
        for i in range(2):
            ld(wo_t[i], wo[i, :, :])

        pOut = ctx.enter_context(tc.tile_pool(name="pOut", bufs=3))
        ob_sb = {}

        def emit_b_chunk(pool, m, ch, copy_engine):
            cs = slice(ch * 512, (ch + 1) * 512)
            pq = pool.tile([P, 512], F32, tag="aux", name=f"pq_{m}_{ch}")
            steps = []
            for kt in range(NKT):
                steps.append(
                    lambda kt=kt, pq=pq: nc.tensor.matmul(
                        pq,
                        lhsT=wqk_t[kt][:, m * P : (m + 1) * P],
                        rhs=xa_t[kt][:, cs],
                        start=(kt == 0),
                        stop=(kt == NKT - 1 and m < 2),
                    )
                )
            if m >= 2:
                steps.append(
                    lambda pq=pq: nc.tensor.matmul(
                        pq,
                        lhsT=kb_t[:, (m - 2) * P : (m - 1) * P],
                        rhs=ak_sb[:, cs],
                        start=False,
                        stop=True,
                    )
                )
                steps.append(
                    lambda pq=pq: copy_engine()(kp2_sb[m - 2][:, cs], pq)
                )
            else:
                def qevac(pq=pq, m=m, cs=cs):
                    # Q bias added during evac: per-partition scalar (DVE)
                    nc.vector.tensor_scalar(
                        qk_sb[m][:, cs], pq, qb_t[:, m : m + 1], None,
                        AluOpType.add,
                    )
                steps.append(qevac)
            return steps

        def emit_c_chunk(pool, mt, copy_engine):
            ms = slice(mt * P, (mt + 1) * P)
            pv = pool.tile([P, HPC * VW], F32, tag="aux", name=f"pv_{mt}")
            steps = []
            for kt in range(NKT):
                steps.append(
                    lambda kt=kt, pv=pv: nc.tensor.matmul(
                        pv,
                        lhsT=xa_t[kt][:, ms],
                        rhs=wv_t[kt],
                        start=(kt == 0),
                        stop=False,
                    )
                )
            steps.append(
                lambda pv=pv: nc.tensor.matmul(
                    pv, lhsT=av_sb[:, ms], rhs=vb_t, start=False, stop=True
                )
            )
            steps.append(lambda pv=pv: copy_engine()(v_sb[mt], pv))
            return steps

        def emit_outproj_chunk(pool, mt, ch, copy_engine):
            ms = slice(mt * P, (mt + 1) * P)
            cs = slice(ch * 512, (ch + 1) * 512)
            po2 = pool.tile([P, 512], F32, tag="aux", name=f"po2_{mt}_{ch}")
            steps = []
            if ch == 0:
                def mkob(mt=mt):
                    ob_sb[mt] = pOut.tile([P, D], BF, tag="ob", name=f"ob_{mt}")
                steps.append(mkob)
            for kt2 in range(2):
                steps.append(
                    lambda kt2=kt2, po2=po2: nc.tensor.matmul(
                        po2,
                        lhsT=oT_sb[kt2][:, ms],
                        rhs=wo_t[kt2][:, cs],
                        start=(kt2 == 0),
                        stop=(kt2 == 1),
                    )
                )

            def fin(po2=po2, mt=mt, ch=ch):
                copy_engine()(ob_sb[mt][:, cs], po2)
                if ch == 1:
                    nc.scalar.dma_start(out=out[mt, 0:64, :], in_=ob_sb[mt][0:64, :])
                    nc.sync.dma_start(out=out[mt, 64:128, :], in_=ob_sb[mt][64:128, :])

            steps.append(fin)
            return steps

        def mk_alt():
            i = [0]

            def pick():
                i[0] += 1
                return nc.vector.tensor_copy if i[0] % 2 else nc.scalar.copy

            return pick

        pro = mk_alt()

        # Phase A (col-tiled pairs) + K^T(m2) kt-major: each k-tile of every
        # accumulation chain is consumed as soon as its DMA lands.
        with tc.tile_pool(name="pA", bufs=1, space="PSUM") as pA, \
             tc.tile_pool(name="pPro", bufs=4, space="PSUM") as pPro:
            pa2 = [pA.tile([P, 512], F32, tag=f"pa{i}", name=f"pa{i}") for i in range(2)]
            bch = [emit_b_chunk(pPro, 2, ch, pro) for ch in range(4)]
            for kt in range(NKT):
                for i in range(2):
                    # cols 2i / 2i+1 at out partitions 0:48 and 64:112 run
                    # concurrently (PE column groups)
                    nc.tensor.matmul(
                        pa2[i][0:48, :],
                        lhsT=ab_t[kt],
                        rhs=xa_t[kt][:, (2 * i) * 512 : (2 * i + 1) * 512],
                        start=(kt == 0),
                        stop=(kt == NKT - 1),
                    )
                    nc.tensor.matmul(
                        pa2[i][64:112, :],
                        lhsT=ab_t[kt],
                        rhs=xa_t[kt][:, (2 * i + 1) * 512 : (2 * i + 2) * 512],
                        start=(kt == 0),
                        stop=(kt == NKT - 1),
                    )
                for c in bch:
                    c[kt]()
            for i in range(2):
                for (r0, dst) in ((0, ak_sb), (32, av_sb)):
                    cs0 = slice((2 * i) * 512, (2 * i + 1) * 512)
                    cs1 = slice((2 * i + 1) * 512, (2 * i + 2) * 512)
                    pro()(dst[0:R, cs0], pa2[i][r0 : r0 + R, :])
                    pro()(dst[0:R, cs1], pa2[i][64 + r0 : 64 + r0 + R, :])
            for c in bch:
                for step in c[NKT:]:
                    step()
        with tc.tile_pool(name="pPro2", bufs=3, space="PSUM") as pPro2:
            for ch in range(2):
                for step in emit_b_chunk(pPro2, 0, ch, pro):
                    step()
            for mt in range(NTT):
                for step in emit_c_chunk(pPro2, mt, pro):
                    step()

        # Attention: 4 head-pair units, each in its own PSUM scope so the
        # score pipeline gets 4 banks (all four K=64 score matmuls of a tj
        # issue back-to-back with no buffer stall) and the PE stream inside
        # a unit switches tiling mode only twice per tj (scores<->PV).
        # B/C/out-proj chunks run in dedicated blocks between units.
        pP = ctx.enter_context(tc.tile_pool(name="pP", bufs=8))
        pEv = ctx.enter_context(tc.tile_pool(name="pEv", bufs=3))
        pN = ctx.enter_context(tc.tile_pool(name="pN", bufs=4))
        pD = ctx.enter_context(tc.tile_pool(name="pD", bufs=3, space="DRAM"))
        alt = mk_alt()

        def emit_unit(half, hp):
            with (
                tc.tile_pool(name=f"pS_{half}_{hp}", bufs=4, space="PSUM") as pS,
                tc.tile_pool(name=f"pO_{half}_{hp}", bufs=4, space="PSUM") as pO,
            ):
                po = {
                    (hl, c): pO.tile([VW, 512], F32, tag="po",
                                     name=f"po_{half}_{hp}_{hl}_{c}")
                    for hl in range(2) for c in range(2)
                }
                pts = {}

                def emit_pv(t, c):
                    for hl in range(2):
                        h = 2 * hp + hl
                        nc.tensor.matmul(
                            po[(hl, c)],
                            lhsT=v_sb[t][:, h * VW : (h + 1) * VW],
                            rhs=pts[t][hl][:, c * 512 : (c + 1) * 512],
                            start=(t == 0),
                            stop=(t == NTT - 1),
                        )

                for tj in range(NTT):
                    pt = [
                        pP.tile([P, HF], BF, tag="pt", name=f"pt_{half}_{hp}_{tj}_{hl}")
                        for hl in range(2)
                    ]
                    tjs = slice(tj * P, (tj + 1) * P)
                    s4 = {}
                    for c in range(2):
                        qcs = slice(half * HF + c * 512, half * HF + (c + 1) * 512)
                        for hl in range(2):
                            rs = slice(hl * HD, (hl + 1) * HD)
                            s_ps = pS.tile([P, 512], F32, tag="s",
                                           name=f"ps_{half}_{hp}_{tj}_{c}_{hl}")
                            nc.tensor.matmul(
                                s_ps,
                                lhsT=kp2_sb[hp][rs, tjs],
                                rhs=qk_sb[hp][rs, qcs],
                                start=True,
                                stop=True,
                            )
                            s4[(c, hl)] = s_ps
                    for c in range(2):
                        act_hl = (tj + c) % 2
                        dve_hl = 1 - act_hl
                        cslice = slice(c * 512, (c + 1) * 512)
                        nc.scalar.activation(
                            pt[act_hl][:, cslice], s4[(c, act_hl)],
                            mybir.ActivationFunctionType.Exp,
                        )
                        nc.vector.tensor_scalar(
                            pt[dve_hl][:, cslice].bitcast(I16), s4[(c, dve_hl)],
                            EXP_A, EXP_B, AluOpType.mult, AluOpType.add,
                        )
                    if tj >= 2:
                        emit_pv(tj - 2, 0)
                        emit_pv(tj - 2, 1)
                    pts[tj] = pt
                    pts.pop(tj - 3, None)
                for t in (NTT - 2, NTT - 1):
                    for c in range(2):
                        emit_pv(t, c)
                # evacuate po before the unit scope closes
                ev = [
                    pEv.tile([VW, HF], BF, tag="ev", name=f"ev_{half}_{hp}_{hl}")
                    for hl in range(2)
                ]
                cp = mk_alt()
                for hl in range(2):
                    for c in range(2):
                        cp()(ev[hl][:, c * 512 : (c + 1) * 512], po[(hl, c)])
            return ev

        def emit_norm_rest(half, hp, ev, last=False):
            hs = slice(half * HF, (half + 1) * HF)
            for hl in range(2):
                den128 = pN.tile([P, HF // P], BF, tag="d128",
                                 name=f"d128_{half}_{hp}_{hl}")
                nc.sync.dma_start(out=den128, in_=ev[hl][HD:VW, :])
                rec = pN.tile([P, HF // P], BF, tag="rec",
                              name=f"rec_{half}_{hp}_{hl}")
                with nc.allow_low_precision(
                    reason="softmax denom ~2048; bf16 recip ~0.4% noise"
                ):
                    nc.vector.reciprocal(rec, den128)
                rw = pD.tile([1, HF], BF, tag="rw", name=f"rw_{half}_{hp}_{hl}")
                nc.sync.dma_start(
                    out=bass.AP(tensor=rw.tensor, offset=rw.offset,
                                ap=[[HF // P, P], [1, HF // P]]),
                    in_=rec,
                )
                rb = pN.tile([HD, HF], BF, tag="rb", name=f"rb_{half}_{hp}_{hl}")
                nsplit = 4 if last else 2
                for sp in range(nsplit):
                    rows = HD // nsplit
                    nc.sync.dma_start(
                        out=rb[sp * rows : (sp + 1) * rows, :],
                        in_=bass.AP(tensor=rw.tensor, offset=rw.offset,
                                    ap=[[0, rows], [1, HF]]),
                    )
                dst = oT_sb[hp][hl * HD : (hl + 1) * HD, hs]
                if hl == 0 and not last:
                    nc.gpsimd.tensor_tensor(dst, ev[hl][0:HD, :], rb,
                                            AluOpType.mult)
                else:
                    nc.vector.tensor_tensor(dst, ev[hl][0:HD, :], rb,
                                            AluOpType.mult)

        def emit_block(name, chunk_lists):
            with tc.tile_pool(name=name, bufs=4, space="PSUM") as pB:
                for mk in chunk_lists:
                    for step in mk(pB):
                        step()

        # unit (0,0); then kp2[1] + Q half-0 of heads 2,3; unit (0,1);
        # then Q half-1; unit (1,0); then out-proj half-0; unit (1,1)
        ev00 = emit_unit(0, 0)
        emit_norm_rest(0, 0, ev00)
        emit_block("pB1", [lambda pB, m=m, ch=ch: emit_b_chunk(pB, m, ch, alt)
                           for (m, ch) in [(3, 0), (3, 1), (3, 2), (3, 3),
                                           (1, 0), (1, 1)]])
        ev01 = emit_unit(0, 1)
        emit_norm_rest(0, 1, ev01)
        emit_block("pB2", [lambda pB, m=m, ch=ch: emit_b_chunk(pB, m, ch, alt)
                           for (m, ch) in [(0, 2), (0, 3), (1, 2), (1, 3)]])
        ev10 = emit_unit(1, 0)
        emit_norm_rest(1, 0, ev10)
        last_ev = emit_unit(1, 1)
        # out-proj half-0 runs AFTER the last unit, bridging the last
        # norm's DMA latency (norm emitted 2 chunks in)
        with tc.tile_pool(name="pB3", bufs=4, space="PSUM") as pB3:
            for i, (mt, ch) in enumerate([(mt, ch) for mt in range(NTT // 2)
                                          for ch in range(2)]):
                for step in emit_outproj_chunk(pB3, mt, ch, alt):
                    step()
                if i == 1:
                    emit_norm_rest(1, 1, last_ev, last=True)

        # Epilogue: half-1 out-projection in two 8-chunk waves; the last
        # unit's norm lands between wave 0's kt2 passes so the kt2=1 matmuls
        # bridge its DMA latency.
        with tc.tile_pool(name="pE", bufs=8, space="PSUM") as pE:
            ecp = mk_alt()
            for wave in range(2):
                mts = range(NTT // 2 + wave * 4, NTT // 2 + wave * 4 + 4)
                chunks = [(mt, ch) for mt in mts for ch in range(2)]
                po2s = {
                    (mt, ch): pE.tile([P, 512], F32, tag="aux", name=f"po2e_{mt}_{ch}")
                    for (mt, ch) in chunks
                }
                for (mt, ch) in chunks:
                    nc.tensor.matmul(
                        po2s[(mt, ch)],
                        lhsT=oT_sb[0][:, mt * P : (mt + 1) * P],
                        rhs=wo_t[0][:, ch * 512 : (ch + 1) * 512],
                        start=True,
                        stop=False,
                    )
                for mt in mts:
                    for ch in range(2):
                        nc.tensor.matmul(
                            po2s[(mt, ch)],
                            lhsT=oT_sb[1][:, mt * P : (mt + 1) * P],
                            rhs=wo_t[1][:, ch * 512 : (ch + 1) * 512],
                            start=False,
                            stop=True,
                        )
                    ob_sb[mt] = pOut.tile([P, D], BF, tag="ob", name=f"ob_{mt}")
                    for ch in range(2):
                        ecp()(ob_sb[mt][:, ch * 512 : (ch + 1) * 512], po2s[(mt, ch)])
                    for p4 in range(4):
                        ps = slice(p4 * 32, (p4 + 1) * 32)
                        (nc.sync if p4 % 2 else nc.scalar).dma_start(
                            out=out[mt, ps, :], in_=ob_sb[mt][ps, :]
                        )

    import bass_rust as _bass_rust

    _bass_rust.move_matmul_waits_to_ldweights(nc.m)
    _bass_rust.generate_event_semaphores(nc)
    return nc


def prepare_in_maps(inputs):
    q = np.asarray(inputs["query"], np.float32)
    ipw = np.asarray(inputs["in_proj_weight"], np.float32)
    ipb = np.asarray(inputs["in_proj_bias"], np.float32)
    out_w = np.asarray(inputs["out_w"], np.float32)
    k_a = np.asarray(inputs["k_a"], np.float32)
    k_b = np.asarray(inputs["k_b"], np.float32)
    v_a = np.asarray(inputs["v_a"], np.float32)
    v_b = np.asarray(inputs["v_b"], np.float32)
    qscale = 1.0 / math.sqrt(HD)
    sl = SCALE / R

    in_maps = []
    for c in range(NCORES):
        bb = c // 4
        s = (c % 4) * CD
        e = s + CD
        X = q[:, bb, :]

        xa = X.T.copy()  # [1024, 2048]

        wqk = np.zeros((D, 2 * CD), np.float32)
        wqk[:, :CD] = ipw[s:e].T * qscale
        wqk[:, CD:] = ipw[D + s : D + e].T

        qbm = (ipb[s:e] * qscale).reshape(2, P)

        wv = np.zeros((D, HPC * VW), np.float32)
        for j in range(HPC):
            wv[:, j * VW : j * VW + HD] = ipw[2 * D + s + j * HD : 2 * D + s + (j + 1) * HD].T

        ab = np.zeros((D, 3 * R), np.float32)
        ab[:, :R] = k_a.T
        ab[:, 2 * R :] = v_a.T

        kbm = np.zeros((R + 1, CD), np.float32)
        kbm[:R] = k_b[:, s:e] * sl
        kbm[R] = ipb[D + s : D + e]

        vbm = np.zeros((R + 1, HPC * VW), np.float32)
        for j in range(HPC):
            vbm[:R, j * VW : j * VW + HD] = v_b[:, s + j * HD : s + (j + 1) * HD] * sl
            vbm[R, j * VW : j * VW + HD] = ipb[2 * D + s + j * HD : 2 * D + s + (j + 1) * HD]
            vbm[R, j * VW + HD] = 1.0

        wo = out_w[:, s:e].T

        in_maps.append(
            {
                "xa": xa.astype(BF16).reshape(NKT, P, T),
                "wqk": wqk.astype(BF16).reshape(NKT, P, 2 * CD),
                "wv": wv.astype(BF16).reshape(NKT, P, HPC * VW),
                "ab": ab.astype(BF16).reshape(NKT, P, 3 * R),
                "qb": qbm.astype(np.float32),
                "kbm": kbm.astype(BF16),
                "vbm": vbm.astype(BF16),
                "wo": wo.astype(BF16).reshape(2, P, D),
            }
        )
    return in_maps


def assemble_output(inputs, results):
    out_b = np.asarray(inputs["out_b"], np.float32)
    out = np.zeros((T, BSZ, D), np.float32)
    for c in range(NCORES):
        out[:, c // 4, :] += results[c]["out"].astype(np.float32).reshape(T, D)
    out += out_b[None, None, :]
    return out


def kernel(**inputs):
    nc = build_nc()
    in_maps = prepare_in_maps(inputs)
    res = run_bass_kernel_spmd(nc, in_maps, core_ids=list(range(NCORES)))
    return assemble_output(inputs, res.results)


# revision 17
# speedup vs baseline: 1.0083x; 1.0083x over previous
"""LoRA MultiheadAttention on 8 NeuronCores (Bass/Tile), v6.

Sharding: 32 (batch, head) attention slices -> 4 heads x 1 batch per core.
Cores 0-3 take batch 0, cores 4-7 batch 1; core c handles heads
(c%4)*4 .. (c%4)*4+3, i.e. a contiguous 256-wide slice of the head dims.

The PE drains one 128-row PSUM column per cycle regardless of K/M/dtype
(fp8 DoubleRow only doubles K per instruction; measured, not 2x drain),
so v6 cuts WALL time with PE tile concurrency (measured ~2x on HW):

  scores:  K=64 per head -> row-group pairs. kp2[hp] stacks head 2hp at
           partitions 0:64 and head 2hp+1 at 64:128; the two K=64 matmuls
           (tile_position (0,0)/(64,0)) run concurrently: 2x score rate.
  LoRA A:  M=48 -> column-group pairs at out partitions 0:48 / 64:112.
  bias folds: no ones row in X^T (8 k-tiles, not 9). K/V biases ride the
           LoRA extra matmul (bias row 16 of kb/vb against an all-ones
           row 16 of ak/av); V's softmax-denominator ones column also
           comes from that row. Q bias is added during PSUM->SBUF evac
           (per-partition bias via ACT activation / DVE tensor_scalar).
  exp:     per (head-pair, tj): 4 [128,512] chunks; ACT does the real
           exp of one head, DVE the Schraudolph bf16 exp of the other
           (i16 = rint(s*128/ln2 + B) bitcast bf16), swapping heads
           every chunk for noise spreading. ~50% Schraudolph.
  norm:    denominator row -> [128,8] via DMA, 128-lane reciprocal,
           DRAM reshape, stride-0 broadcast; the even head's oT multiply
           runs on GpSimd (SBUF-only engine, otherwise idle), the odd
           head's on DVE (needs the partition shift).
  filler:  B (Q^T/K^T), C (V row-tiles) and the half-0 out-projection
           drain into the attention PE stream; epilogue does half-1 in
           two 8-chunk waves with the last norm between kt2 passes.
  head/tail: first k-tile's DMAs split across rings and dispatched from
           both Sync and ACT queues; output DMAs partition-split.

b_v/b_k fold into LoRA rows; out_b added on host.
"""

import sys

sys.path.insert(0, "/opt/trn_rl_repo")

import math
from contextlib import ExitStack

import ml_dtypes
import numpy as np

import concourse.bass as bass
import concourse.tile as tile
from concourse import mybir
from concourse.alu_op_type import AluOpType
from concourse.bass_utils import run_bass_kernel_spmd

BF16 = ml_dtypes.bfloat16
F32 = mybir.dt.float32
BF = mybir.dt.bfloat16
I16 = mybir.dt.int16

T = 2048
D = 1024
H = 16
HD = 64
R = 16
BSZ = 2
SCALE = 16.0
NCORES = 8
HPC = 4  # heads per core
CD = HPC * HD  # 256 head dims per core
VW = HD + 1  # V block width per head (ones column appended)
NKT = 8  # 1024 contraction rows = 8 k-tiles (biases folded elsewhere)
P = 128
NTT = T // P  # 16 row tiles
HF = T // 2  # 1024: queries processed in two halves

# Schraudolph-bf16 exp: i16 = rint(x * 128/ln2 + (127*128 - C)), bitcast bf16
EXP_A = 128.0 / math.log(2.0)
EXP_B = 127.0 * 128.0 - 7.3


def build_nc():
    nc = bass.Bass()
    xa = nc.dram_tensor("xa", [NKT, P, T], BF, kind="ExternalInput")
    wqk = nc.dram_tensor("wqk", [NKT, P, 2 * CD], BF, kind="ExternalInput")
    wv = nc.dram_tensor("wv", [NKT, P, HPC * VW], BF, kind="ExternalInput")
    ab = nc.dram_tensor("ab", [NKT, P, 3 * R], BF, kind="ExternalInput")
    qb = nc.dram_tensor("qb", [2, P], F32, kind="ExternalInput")
    kbm = nc.dram_tensor("kbm", [R + 1, CD], BF, kind="ExternalInput")
    vbm = nc.dram_tensor("vbm", [R + 1, HPC * VW], BF, kind="ExternalInput")
    wo = nc.dram_tensor("wo", [2, P, D], BF, kind="ExternalInput")
    out = nc.dram_tensor("out", [NTT, P, D], BF, kind="ExternalOutput")

    with tile.TileContext(nc) as tc, ExitStack() as ctx:
        singles = ctx.enter_context(tc.tile_pool(name="singles", bufs=1))

        xa_t = [singles.tile([P, T], BF, name=f"xa{i}", tag=f"xa{i}") for i in range(NKT)]
        wqk_t = [singles.tile([P, 2 * CD], BF, name=f"wqk{i}", tag=f"wqk{i}") for i in range(NKT)]
        wv_t = [singles.tile([P, HPC * VW], BF, name=f"wv{i}", tag=f"wv{i}") for i in range(NKT)]
        ab_t = [singles.tile([P, 3 * R], BF, name=f"ab{i}", tag=f"ab{i}") for i in range(NKT)]
        qb_t = singles.tile([P, 2], F32, tag="qb")
        kb_t = singles.tile([P, CD], BF, tag="kb")
        vb_t = singles.tile([P, HPC * VW], BF, tag="vb")
        ak_sb = singles.tile([P, T], BF, tag="ak")
        av_sb = singles.tile([P, T], BF, tag="av")
        qk_sb = [singles.tile([P, T], BF, name=f"qk{i}", tag=f"qk{i}") for i in range(2)]
        kp2_sb = [singles.tile([P, T], BF, name=f"kp{i}", tag=f"kp{i}") for i in range(2)]
        v_sb = [singles.tile([P, HPC * VW], BF, name=f"v{i}", tag=f"v{i}") for i in range(NTT)]
        oT_sb = [singles.tile([P, T], BF, name=f"oT{i}", tag=f"oT{i}") for i in range(2)]
        wo_t = [singles.tile([P, D], BF, name=f"wo{i}", tag=f"wo{i}") for i in range(2)]

        # memsets first: DVE is idle while the DMA queues spin up.
        # ak/av zero except the all-ones row at partition 32 that applies
        # the K/V bias row 32 of kb/vb (32-aligned engine partition base).
        nc.vector.memset(ak_sb, 0.0)
        nc.vector.memset(ak_sb[32:33, :], 1.0)
        nc.vector.memset(av_sb, 0.0)
        nc.vector.memset(av_sb[32:33, :], 1.0)
        nc.gpsimd.memset(kb_t, 0.0)
        nc.gpsimd.memset(vb_t, 0.0)

        # input loads: first k-tile split across rings, alternating the two
        # HW-DGE dispatch queues (Sync + ACT) so descriptors land ~2x faster
        eng_i = [0]

        def ld(dst, src):
            eng_i[0] += 1
            (nc.sync if eng_i[0] % 2 else nc.scalar).dma_start(out=dst, in_=src)

        for p4 in range(4):
            ps = slice(p4 * 32, (p4 + 1) * 32)
            ld(ab_t[0][ps, :], ab[0, ps, :])
        for p2 in range(2):
            ps = slice(p2 * 64, (p2 + 1) * 64)
            ld(xa_t[0][ps, 0:512], xa[0, ps, 0:512])
        for p2 in range(2):
            ps = slice(p2 * 64, (p2 + 1) * 64)
            ld(wqk_t[0][ps, :], wqk[0, ps, :])
        # qb arrives transposed [2,128] -> [128,2] via AP swap (tiny but
        # descriptor-heavy; dispatch early, needed by the first Q evac)
        ld(qb_t, qb[:, :].rearrange("a b -> b a"))
        for c4 in range(1, 4):
            cs = slice(c4 * 512, (c4 + 1) * 512)
            ld(xa_t[0][:, cs], xa[0, :, cs])
        for i in range(1, NKT):
            ld(ab_t[i], ab[i, :, :])
            ld(xa_t[i], xa[i, :, :])
            ld(wqk_t[i], wqk[i, :, :])
        ld(kb_t[0:R, :], kbm[0:R, :])
        ld(kb_t[32:33, :], kbm[R : R + 1, :])
        ld(vb_t[0:R, :], vbm[0:R, :])
        ld(vb_t[32:33, :], vbm[R : R + 1, :])
        for i in range(NKT):
            ld(wv_t[i], wv[i, :, :])
        for i in range(2):
            ld(wo_t[i], wo[i, :, :])

        pOut = ctx.enter_context(tc.tile_pool(name="pOut", bufs=3))
        ob_sb = {}

        def emit_b_chunk(pool, m, ch, copy_engine):
            cs = slice(ch * 512, (ch + 1) * 512)
            pq = pool.tile([P, 512], F32, tag="aux", name=f"pq_{m}_{ch}")
            steps = []
            for kt in range(NKT):
                steps.append(
                    lambda kt=kt, pq=pq: nc.tensor.matmul(
                        pq,
                        lhsT=wqk_t[kt][:, m * P : (m + 1) * P],
                        rhs=xa_t[kt][:, cs],
                        start=(kt == 0),
                        stop=(kt == NKT - 1 and m < 2),
                    )
                )
            if m >= 2:
                steps.append(
                    lambda pq=pq: nc.tensor.matmul(
                        pq,
                        lhsT=kb_t[:, (m - 2) * P : (m - 1) * P],
                        rhs=ak_sb[:, cs],
                        start=False,
                        stop=True,
                    )
                )
                steps.append(
                    lambda pq=pq: copy_engine()(kp2_sb[m - 2][:, cs], pq)
                )
            else:
                def qevac(pq=pq, m=m, cs=cs):
                    # Q bias added during evac: per-partition scalar (DVE)
                    nc.vector.tensor_scalar(
                        qk_sb[m][:, cs], pq, qb_t[:, m : m + 1], None,
                        AluOpType.add,
                    )
                steps.append(qevac)
            return steps

        def emit_c_chunk(pool, mt, copy_engine):
            ms = slice(mt * P, (mt + 1) * P)
            pv = pool.tile([P, HPC * VW], F32, tag="aux", name=f"pv_{mt}")
            steps = []
            for kt in range(NKT):
                steps.append(
                    lambda kt=kt, pv=pv: nc.tensor.matmul(
                        pv,
                        lhsT=xa_t[kt][:, ms],
                        rhs=wv_t[kt],
                        start=(kt == 0),
                        stop=False,
                    )
                )
            steps.append(
                lambda pv=pv: nc.tensor.matmul(
                    pv, lhsT=av_sb[:, ms], rhs=vb_t, start=False, stop=True
                )
            )
            steps.append(lambda pv=pv: copy_engine()(v_sb[mt], pv))
            return steps

        def emit_outproj_chunk(pool, mt, ch, copy_engine):
            ms = slice(mt * P, (mt + 1) * P)
            cs = slice(ch * 512, (ch + 1) * 512)
            po2 = pool.tile([P, 512], F32, tag="aux", name=f"po2_{mt}_{ch}")
            steps = []
            if ch == 0:
                def mkob(mt=mt):
                    ob_sb[mt] = pOut.tile([P, D], BF, tag="ob", name=f"ob_{mt}")
                steps.append(mkob)
            for kt2 in range(2):
                steps.append(
                    lambda kt2=kt2, po2=po2: nc.tensor.matmul(
                        po2,
                        lhsT=oT_sb[kt2][:, ms],
                        rhs=wo_t[kt2][:, cs],
                        start=(kt2 == 0),
                        stop=(kt2 == 1),
                    )
                )

            def fin(po2=po2, mt=mt, ch=ch):
                copy_engine()(ob_sb[mt][:, cs], po2)
                if ch == 1:
                    nc.scalar.dma_start(out=out[mt, 0:64, :], in_=ob_sb[mt][0:64, :])
                    nc.sync.dma_start(out=out[mt, 64:128, :], in_=ob_sb[mt][64:128, :])

            steps.append(fin)
            return steps

        def mk_alt():
            i = [0]

            def pick():
                i[0] += 1
                return nc.vector.tensor_copy if i[0] % 2 else nc.scalar.copy

            return pick

        pro = mk_alt()

        # Phase A (col-tiled pairs) + K^T(m2) kt-major: each k-tile of every
        # accumulation chain is consumed as soon as its DMA lands.
        with tc.tile_pool(name="pA", bufs=1, space="PSUM") as pA, \
             tc.tile_pool(name="pPro", bufs=4, space="PSUM") as pPro:
            pa2 = [pA.tile([P, 512], F32, tag=f"pa{i}", name=f"pa{i}") for i in range(2)]
            bch = [emit_b_chunk(pPro, 2, ch, pro) for ch in range(4)]
            for kt in range(NKT):
                for i in range(2):
                    # cols 2i / 2i+1 at out partitions 0:48 and 64:112 run
                    # concurrently (PE column groups)
                    nc.tensor.matmul(
                        pa2[i][0:48, :],
                        lhsT=ab_t[kt],
                        rhs=xa_t[kt][:, (2 * i) * 512 : (2 * i + 1) * 512],
                        start=(kt == 0),
                        stop=(kt == NKT - 1),
                    )
                    nc.tensor.matmul(
                        pa2[i][64:112, :],
                        lhsT=ab_t[kt],
                        rhs=xa_t[kt][:, (2 * i + 1) * 512 : (2 * i + 2) * 512],
                        start=(kt == 0),
                        stop=(kt == NKT - 1),
                    )
                for c in bch:
                    c[kt]()
            for i in range(2):
                for (r0, dst) in ((0, ak_sb), (32, av_sb)):
                    cs0 = slice((2 * i) * 512, (2 * i + 1) * 512)
                    cs1 = slice((2 * i + 1) * 512, (2 * i + 2) * 512)
                    pro()(dst[0:R, cs0], pa2[i][r0 : r0 + R, :])
                    pro()(dst[0:R, cs1], pa2[i][64 + r0 : 64 + r0 + R, :])
            for c in bch:
                for step in c[NKT:]:
                    step()
        with tc.tile_pool(name="pPro2", bufs=3, space="PSUM") as pPro2:
            for ch in range(2):
                for step in emit_b_chunk(pPro2, 0, ch, pro):
                    step()
            for mt in range(NTT):
                for step in emit_c_chunk(pPro2, mt, pro):
                    step()

        # Attention: 4 head-pair units, each in its own PSUM scope so the
        # score pipeline gets 4 banks (all four K=64 score matmuls of a tj
        # issue back-to-back with no buffer stall) and the PE stream inside
        # a unit switches tiling mode only twice per tj (scores<->PV).
        # B/C/out-proj chunks run in dedicated blocks between units.
        pP = ctx.enter_context(tc.tile_pool(name="pP", bufs=8))
        pEv = ctx.enter_context(tc.tile_pool(name="pEv", bufs=3))
        pN = ctx.enter_context(tc.tile_pool(name="pN", bufs=4))
        pD = ctx.enter_context(tc.tile_pool(name="pD", bufs=3, space="DRAM"))
        alt = mk_alt()

        def emit_unit(half, hp):
            with (
                tc.tile_pool(name=f"pS_{half}_{hp}", bufs=4, space="PSUM") as pS,
                tc.tile_pool(name=f"pO_{half}_{hp}", bufs=4, space="PSUM") as pO,
            ):
                po = {
                    (hl, c): pO.tile([VW, 512], F32, tag="po",
                                     name=f"po_{half}_{hp}_{hl}_{c}")
                    for hl in range(2) for c in range(2)
                }
                pts = {}

                def emit_pv(t, c):
                    for hl in range(2):
                        h = 2 * hp + hl
                        nc.tensor.matmul(
                            po[(hl, c)],
                            lhsT=v_sb[t][:, h * VW : (h + 1) * VW],
                            rhs=pts[t][hl][:, c * 512 : (c + 1) * 512],
                            start=(t == 0),
                            stop=(t == NTT - 1),
                        )

                for tj in range(NTT):
                    pt = [
                        pP.tile([P, HF], BF, tag="pt", name=f"pt_{half}_{hp}_{tj}_{hl}")
                        for hl in range(2)
                    ]
                    tjs = slice(tj * P, (tj + 1) * P)
                    s4 = {}
                    for c in range(2):
                        qcs = slice(half * HF + c * 512, half * HF + (c + 1) * 512)
                        for hl in range(2):
                            rs = slice(hl * HD, (hl + 1) * HD)
                            s_ps = pS.tile([P, 512], F32, tag="s",
                                           name=f"ps_{half}_{hp}_{tj}_{c}_{hl}")
                            nc.tensor.matmul(
                                s_ps,
                                lhsT=kp2_sb[hp][rs, tjs],
                                rhs=qk_sb[hp][rs, qcs],
                                start=True,
                                stop=True,
                            )
                            s4[(c, hl)] = s_ps
                    for c in range(2):
                        act_hl = (tj + c) % 2
                        dve_hl = 1 - act_hl
                        cslice = slice(c * 512, (c + 1) * 512)
                        nc.scalar.activation(
                            pt[act_hl][:, cslice], s4[(c, act_hl)],
                            mybir.ActivationFunctionType.Exp,
                        )
                        nc.vector.tensor_scalar(
                            pt[dve_hl][:, cslice].bitcast(I16), s4[(c, dve_hl)],
                            EXP_A, EXP_B, AluOpType.mult, AluOpType.add,
                        )
                    if tj >= 2:
                        emit_pv(tj - 2, 0)
                        emit_pv(tj - 2, 1)
                    pts[tj] = pt
                    pts.pop(tj - 3, None)
                for t in (NTT - 2, NTT - 1):
                    for c in range(2):
                        emit_pv(t, c)
                # evacuate po before the unit scope closes
                ev = [
                    pEv.tile([VW, HF], BF, tag="ev", name=f"ev_{half}_{hp}_{hl}")
                    for hl in range(2)
                ]
                cp = mk_alt()
                for hl in range(2):
                    for c in range(2):
                        cp()(ev[hl][:, c * 512 : (c + 1) * 512], po[(hl, c)])
            return ev

        def emit_norm_rest(half, hp, ev, last=False):
            hs = slice(half * HF, (half + 1) * HF)
            for hl in range(2):
                den128 = pN.tile([P, HF // P], BF, tag="d128",
                                 name=f"d128_{half}_{hp}_{hl}")
                nc.sync.dma_start(out=den128, in_=ev[hl][HD:VW, :])
                rec = pN.tile([P, HF // P], BF, tag="rec",
                              name=f"rec_{half}_{hp}_{hl}")
                with nc.allow_low_precision(
                    reason="softmax denom ~2048; bf16 recip ~0.4% noise"
                ):
                    nc.vector.reciprocal(rec, den128)
                rw = pD.tile([1, HF], BF, tag="rw", name=f"rw_{half}_{hp}_{hl}")
                nc.sync.dma_start(
                    out=bass.AP(tensor=rw.tensor, offset=rw.offset,
                                ap=[[HF // P, P], [1, HF // P]]),
                    in_=rec,
                )
                rb = pN.tile([HD, HF], BF, tag="rb", name=f"rb_{half}_{hp}_{hl}")
                nsplit = 4 if last else 2
                for sp in range(nsplit):
                    rows = HD // nsplit
                    nc.sync.dma_start(
                        out=rb[sp * rows : (sp + 1) * rows, :],
                        in_=bass.AP(tensor=rw.tensor, offset=rw.offset,
                                    ap=[[0, rows], [1, HF]]),
                    )
                dst = oT_sb[hp][hl * HD : (hl + 1) * HD, hs]
                if hl == 0 and not last:
                    nc.gpsimd.tensor_tensor(dst, ev[hl][0:HD, :], rb,
                                            AluOpType.mult)
                else:
                    nc.vector.tensor_tensor(dst, ev[hl][0:HD, :], rb,
                                            AluOpType.mult)

        def emit_block(name, chunk_lists):
            with tc.tile_pool(name=name, bufs=4, space="PSUM") as pB:
                for mk in chunk_lists:
                    for step in mk(pB):
                        step()

        # unit (0,0); then kp2[1] + Q half-0 of heads 2,3; unit (0,1);
        # then Q half-1; unit (1,0); then out-proj half-0; unit (1,1)
        ev00 = emit_unit(0, 0)
        emit_norm_rest(0, 0, ev00)
        emit_block("pB1", [lambda pB, m=m, ch=ch: emit_b_chunk(pB, m, ch, alt)
                           for (m, ch) in [(3, 0), (3, 1), (3, 2), (3, 3),
                                           (1, 0), (1, 1)]])
        ev01 = emit_unit(0, 1)
        emit_norm_rest(0, 1, ev01)
        emit_block("pB2", [lambda pB, m=m, ch=ch: emit_b_chunk(pB, m, ch, alt)
                           for (m, ch) in [(0, 2), (0, 3), (1, 2), (1, 3)]])
        ev10 = emit_unit(1, 0)
        emit_norm_rest(1, 0, ev10)
        last_ev = emit_unit(1, 1)
        # out-proj half-0 runs AFTER the last unit, bridging the last
        # norm's DMA latency (norm emitted 2 chunks in)
        with tc.tile_pool(name="pB3", bufs=4, space="PSUM") as pB3:
            for i, (mt, ch) in enumerate([(mt, ch) for mt in range(NTT // 2)
                                          for ch in range(2)]):
                for step in emit_outproj_chunk(pB3, mt, ch, alt):
                    step()
                if i == 1:
                    emit_norm_rest(1, 1, last_ev, last=True)

        # Epilogue: half-1 out-projection in two 8-chunk waves; the last
        # unit's norm lands between wave 0's kt2 passes so the kt2=1 matmuls
        # bridge its DMA latency.
        with tc.tile_pool(name="pE", bufs=8, space="PSUM") as pE:
            ecp = mk_alt()
            for wave in range(2):
                mts = range(NTT // 2 + wave * 4, NTT // 2 + wave * 4 + 4)
                chunks = [(mt, ch) for mt in mts for ch in range(2)]
                po2s = {
                    (mt, ch): pE.tile([P, 512], F32, tag="aux", name=f"po2e_{mt}_{ch}")
                    for (mt, ch) in chunks
                }
                for (mt, ch) in chunks:
                    nc.tensor.matmul(
                        po2s[(mt, ch)],
                        lhsT=oT_sb[0][:, mt * P : (mt + 1) * P],
                        rhs=wo_t[0][:, ch * 512 : (ch + 1) * 512],
                        start=True,
                        stop=False,
                    )
                for mt in mts:
                    for ch in range(2):
                        nc.tensor.matmul(
                            po2s[(mt, ch)],
                            lhsT=oT_sb[1][:, mt * P : (mt + 1) * P],
                            rhs=wo_t[1][:, ch * 512 : (ch + 1) * 512],
                            start=False,
                            stop=True,
                        )
                    ob_sb[mt] = pOut.tile([P, D], BF, tag="ob", name=f"ob_{mt}")
                    for ch in range(2):
                        ecp()(ob_sb[mt][:, ch * 512 : (ch + 1) * 512], po2s[(mt, ch)])
                    for p4 in range(4):
                        ps = slice(p4 * 32, (p4 + 1) * 32)
                        (nc.sync if p4 % 2 else nc.scalar).dma_start(
                            out=out[mt, ps, :], in_=ob_sb[mt][ps, :]
                        )

    import bass_rust as _bass_rust

    _bass_rust.move_matmul_waits_to_ldweights(nc.m)
    _bass_rust.generate_event_semaphores(nc)
    return nc


def prepare_in_maps(inputs):
    q = np.asarray(inputs["query"], np.float32)
    ipw = np.asarray(inputs["in_proj_weight"], np.float32)
    ipb = np.asarray(inputs["in_proj_bias"], np.float32)
    out_w = np.asarray(inputs["out_w"], np.float32)
    k_a = np.asarray(inputs["k_a"], np.float32)
    k_b = np.asarray(inputs["k_b"], np.float32)
    v_a = np.asarray(inputs["v_a"], np.float32)
    v_b = np.asarray(inputs["v_b"], np.float32)
    qscale = 1.0 / math.sqrt(HD)
    sl = SCALE / R

    in_maps = []
    for c in range(NCORES):
        bb = c // 4
        s = (c % 4) * CD
        e = s + CD
        X = q[:, bb, :]

        xa = X.T.copy()  # [1024, 2048]

        wqk = np.zeros((D, 2 * CD), np.float32)
        wqk[:, :CD] = ipw[s:e].T * qscale
        wqk[:, CD:] = ipw[D + s : D + e].T

        qbm = (ipb[s:e] * qscale).reshape(2, P)

        wv = np.zeros((D, HPC * VW), np.float32)
        for j in range(HPC):
            wv[:, j * VW : j * VW + HD] = ipw[2 * D + s + j * HD : 2 * D + s + (j + 1) * HD].T

        ab = np.zeros((D, 3 * R), np.float32)
        ab[:, :R] = k_a.T
        ab[:, 2 * R :] = v_a.T

        kbm = np.zeros((R + 1, CD), np.float32)
        kbm[:R] = k_b[:, s:e] * sl
        kbm[R] = ipb[D + s : D + e]

        vbm = np.zeros((R + 1, HPC * VW), np.float32)
        for j in range(HPC):
            vbm[:R, j * VW : j * VW + HD] = v_b[:, s + j * HD : s + (j + 1) * HD] * sl
            vbm[R, j * VW : j * VW + HD] = ipb[2 * D + s + j * HD : 2 * D + s + (j + 1) * HD]
            vbm[R, j * VW + HD] = 1.0

        wo = out_w[:, s:e].T

        in_maps.append(
            {
                "xa": xa.astype(BF16).reshape(NKT, P, T),
                "wqk": wqk.astype(BF16).reshape(NKT, P, 2 * CD),
                "wv": wv.astype(BF16).reshape(NKT, P, HPC * VW),
                "ab": ab.astype(BF16).reshape(NKT, P, 3 * R),
                "qb": qbm.astype(np.float32),
                "kbm": kbm.astype(BF16),
                "vbm": vbm.astype(BF16),
                "wo": wo.astype(BF16).reshape(2, P, D),
            }
        )
    return in_maps


def assemble_output(inputs, results):
    out_b = np.asarray(inputs["out_b"], np.float32)
    out = np.zeros((T, BSZ, D), np.float32)
    for c in range(NCORES):
        out[:, c // 4, :] += results[c]["out"].astype(np.float32).reshape(T, D)
    out += out_b[None, None, :]
    return out


def kernel(**inputs):
    nc = build_nc()
    in_maps = prepare_in_maps(inputs)
    res = run_bass_kernel_spmd(nc, in_maps, core_ids=list(range(NCORES)))
    return assemble_output(inputs, res.results)


# revision 18
# speedup vs baseline: 1.0799x; 1.0711x over previous
"""LoRA MultiheadAttention on 8 NeuronCores (Bass/Tile), v5.

Sharding: 32 (batch, head) attention slices -> 4 heads x 1 batch per core.
Cores 0-3 take batch 0, cores 4-7 batch 1; core c handles heads
(c%4)*4 .. (c%4)*4+3, i.e. a contiguous 256-wide slice of the head dims.

The PE is drain-bound on TRN2 (every matmul costs N fp32-PSUM-drain columns
at 1 col/cycle regardless of K/M), so the kernel keeps the PE instruction
stream dense end-to-end (all matmuls bf16; fp8 was tried and rejected:
e4m3's ~4% per-element noise does not average away in random GEMMs):

  prologue: A^T LoRA activations, all of Q^T/K^T, V row-tiles 0-2.
  attention: 8 single-head units x 16 tj iterations x 2 512-wide score
             chunks. 2 chunks/iter against 3 pS slots leaves a full exp of
             cross-iteration slack, so score matmuls rarely wait. The
             remaining 13 V row-tiles (paced ahead of their PV consumers)
             and the half-0 out-projection are drained into the PE stream
             as filler so exp waits never idle the PE; filler accumulates
             in a single spare PSUM bank.
  exp split: ACT (real exp) and DVE (one-op Schraudolph bf16:
             i16 = rint(s*128/ln2 + B) bitcast bf16, mean-zero calibrated,
             ~40% of chunks; softmax renormalizes, output err ~0.5%).
  norm:      po evacuated to bf16 SBUF immediately (frees PSUM banks);
             denominator row round-trips through DRAM reshaped to [128, 8]
             so the reciprocal uses 128 DVE lanes (0.13us vs 6.5us for a
             [64,1024] broadcast reciprocal); stride-0 DMA broadcast; one
             2x-mode bf16 multiply into oT_sb.
  epilogue:  half-1 out-projection, PSUM->SBUF copies alternating ACT/DVE.

b_v is folded into the V matmul ones-row bias; out_b added on host.
"""

import sys

sys.path.insert(0, "/opt/trn_rl_repo")

import math
from contextlib import ExitStack

import ml_dtypes
import numpy as np

import concourse.bass as bass
import concourse.tile as tile
from concourse import mybir
from concourse.alu_op_type import AluOpType
from concourse.bass_utils import run_bass_kernel_spmd

BF16 = ml_dtypes.bfloat16
F32 = mybir.dt.float32
BF = mybir.dt.bfloat16
I16 = mybir.dt.int16

T = 2048
D = 1024
H = 16
HD = 64
R = 16
BSZ = 2
SCALE = 16.0
NCORES = 8
HPC = 4  # heads per core
CD = HPC * HD  # 256 head dims per core
VW = HD + 1  # V block width per head (ones column appended)
KPAD = 1152  # 1024 X rows + 1 ones row, padded to 9 k-tiles of 128
NKT = KPAD // 128
P = 128
NTT = T // P  # 16 row tiles
HF = T // 2  # 1024: ti processed in two halves

# Schraudolph-bf16 exp: i16 = rint(x * 128/ln2 + (127*128 - C)), bitcast bf16
EXP_A = 128.0 / math.log(2.0)
EXP_B = 127.0 * 128.0 - 7.3
# tj tiles whose c==1 exp chunk goes to DVE-Schraudolph (rest go to ACT)
DVE_TJ = frozenset(range(16)) - {5, 10, 15}


def build_nc():
    nc = bass.Bass()
    xa = nc.dram_tensor("xa", [NKT, P, T], BF, kind="ExternalInput")
    wqk = nc.dram_tensor("wqk", [NKT, P, 2 * CD], BF, kind="ExternalInput")
    wv = nc.dram_tensor("wv", [NKT, P, HPC * VW], BF, kind="ExternalInput")
    ab = nc.dram_tensor("ab", [NKT, P, 3 * R], BF, kind="ExternalInput")
    kbm = nc.dram_tensor("kbm", [R, CD], BF, kind="ExternalInput")
    vbm = nc.dram_tensor("vbm", [R, HPC * VW], BF, kind="ExternalInput")
    wo = nc.dram_tensor("wo", [2, P, D], BF, kind="ExternalInput")
    out = nc.dram_tensor("out", [NTT, P, D], BF, kind="ExternalOutput")

    with tile.TileContext(nc) as tc, ExitStack() as ctx:
        singles = ctx.enter_context(tc.tile_pool(name="singles", bufs=1))

        xa_t = [singles.tile([P, T], BF, name=f"xa{i}", tag=f"xa{i}") for i in range(NKT)]
        wqk_t = [singles.tile([P, 2 * CD], BF, name=f"wqk{i}", tag=f"wqk{i}") for i in range(NKT)]
        wv_t = [singles.tile([P, HPC * VW], BF, name=f"wv{i}", tag=f"wv{i}") for i in range(NKT)]
        ab_t = [singles.tile([P, 3 * R], BF, name=f"ab{i}", tag=f"ab{i}") for i in range(NKT)]
        kb_t = singles.tile([P, CD], BF, tag="kb")
        vb_t = singles.tile([P, HPC * VW], BF, tag="vb")
        nc.vector.memset(kb_t, 0.0)
        nc.vector.memset(vb_t, 0.0)
        wo_t = [singles.tile([P, D], BF, name=f"wo{i}", tag=f"wo{i}") for i in range(2)]
        # load order matches consumption: A needs ab+xa, then B needs wqk
        for c4 in range(4):
            nc.sync.dma_start(
                out=xa_t[0][:, c4 * 512 : (c4 + 1) * 512],
                in_=xa[0, :, c4 * 512 : (c4 + 1) * 512],
            )
            if c4 == 0:
                nc.sync.dma_start(out=ab_t[0], in_=ab[0, :, :])
        nc.sync.dma_start(out=wqk_t[0], in_=wqk[0, :, :])
        for i in range(1, NKT):
            nc.sync.dma_start(out=ab_t[i], in_=ab[i, :, :])
            nc.sync.dma_start(out=xa_t[i], in_=xa[i, :, :])
            nc.sync.dma_start(out=wqk_t[i], in_=wqk[i, :, :])
        for i in range(NKT):
            nc.sync.dma_start(out=wv_t[i], in_=wv[i, :, :])
        nc.sync.dma_start(out=kb_t[0:R, :], in_=kbm[:, :])
        nc.sync.dma_start(out=vb_t[0:R, :], in_=vbm[:, :])
        for i in range(2):
            nc.sync.dma_start(out=wo_t[i], in_=wo[i, :, :])

        # Q^T tiles (heads 0-1 / 2-3); K^T stored per head zero-padded to
        # 128 contraction rows so every attention matmul runs in the PE's
        # (128,128) tiling mode -- mode switches drain the whole array.
        qk_sb = [singles.tile([P, T], BF, name=f"qk{i}", tag=f"qk{i}") for i in range(2)]
        kp_sb = [singles.tile([P, T], BF, name=f"kp{i}", tag=f"kp{i}") for i in range(HPC)]
        ak_sb = singles.tile([P, T], BF, tag="ak")
        av_sb = singles.tile([P, T], BF, tag="av")
        for t8 in kp_sb:
            nc.vector.memset(t8, 0.0)
        nc.vector.memset(ak_sb, 0.0)
        nc.vector.memset(av_sb, 0.0)
        v_sb = [singles.tile([P, HPC * VW], BF, name=f"v{i}", tag=f"v{i}") for i in range(NTT)]
        oT_sb = [singles.tile([P, T], BF, name=f"oT{i}", tag=f"oT{i}") for i in range(2)]
        pOut = ctx.enter_context(tc.tile_pool(name="pOut", bufs=3))
        ob_sb = {}

        def emit_b_chunk(pool, m, ch, copy_engine):
            cs = slice(ch * 512, (ch + 1) * 512)
            pq = pool.tile([P, 512], F32, tag="aux", name=f"pq_{m}_{ch}")
            steps = []
            for kt in range(NKT):
                steps.append(
                    lambda kt=kt, pq=pq: nc.tensor.matmul(
                        pq,
                        lhsT=wqk_t[kt][:, m * P : (m + 1) * P],
                        rhs=xa_t[kt][:, cs],
                        start=(kt == 0),
                        stop=(kt == NKT - 1 and m < 2),
                    )
                )
            if m >= 2:
                steps.append(
                    lambda pq=pq: nc.tensor.matmul(
                        pq,
                        lhsT=kb_t[:, (m - 2) * P : (m - 1) * P],
                        rhs=ak_sb[:, cs],
                        start=False,
                        stop=True,
                    )
                )
                h0 = 2 * (m - 2)
                steps.append(
                    lambda pq=pq, h0=h0: copy_engine(
                        kp_sb[h0][0:HD, cs], pq[0:HD, :]
                    )
                )
                steps.append(
                    lambda pq=pq, h0=h0: copy_engine(
                        kp_sb[h0 + 1][HD:P, cs], pq[HD:P, :]
                    )
                )
            else:
                steps.append(lambda pq=pq: copy_engine(qk_sb[m][:, cs], pq))
            return steps

        def emit_c_chunk(pool, mt, copy_engine):
            ms = slice(mt * P, (mt + 1) * P)
            pv = pool.tile([P, HPC * VW], F32, tag="aux", name=f"pv_{mt}")
            steps = []
            for kt in range(NKT):
                steps.append(
                    lambda kt=kt, pv=pv: nc.tensor.matmul(
                        pv,
                        lhsT=xa_t[kt][:, ms],
                        rhs=wv_t[kt],
                        start=(kt == 0),
                        stop=False,
                    )
                )
            steps.append(
                lambda pv=pv: nc.tensor.matmul(
                    pv, lhsT=av_sb[:, ms], rhs=vb_t, start=False, stop=True
                )
            )
            steps.append(lambda pv=pv: copy_engine(v_sb[mt], pv))
            return steps

        def emit_outproj_chunk(pool, mt, ch, copy_engine):
            ms = slice(mt * P, (mt + 1) * P)
            cs = slice(ch * 512, (ch + 1) * 512)
            po2 = pool.tile([P, 512], F32, tag="aux", name=f"po2_{mt}_{ch}")
            steps = []
            if ch == 0:
                def mkob(mt=mt):
                    ob_sb[mt] = pOut.tile([P, D], BF, tag="ob", name=f"ob_{mt}")
                steps.append(mkob)
            for kt2 in range(2):
                steps.append(
                    lambda kt2=kt2, po2=po2: nc.tensor.matmul(
                        po2,
                        lhsT=oT_sb[kt2][:, ms],
                        rhs=wo_t[kt2][:, cs],
                        start=(kt2 == 0),
                        stop=(kt2 == 1),
                    )
                )

            def fin(po2=po2, mt=mt, ch=ch):
                copy_engine(ob_sb[mt][:, cs], po2)
                if ch == 1:
                    nc.sync.dma_start(out=out[mt, :, :], in_=ob_sb[mt])

            steps.append(fin)
            return steps

        def pro_copy(i=[0]):
            i[0] += 1
            return nc.vector.tensor_copy if i[0] % 2 else nc.scalar.copy

        # Phase A + K^T(m2) prologue, kt-major: each k-tile of every
        # accumulation chain is consumed as soon as its DMA lands, so the
        # PE tracks the input load instead of waiting for it.
        with tc.tile_pool(name="pA", bufs=4, space="PSUM") as pA, \
             tc.tile_pool(name="pPro", bufs=4, space="PSUM") as pPro:
            pa4 = [pA.tile([3 * R, 512], F32, tag="pa", name=f"pa{ch}") for ch in range(4)]
            bch = [emit_b_chunk(pPro, 2, ch, pro_copy()) for ch in range(4)]
            for kt in range(NKT):
                if kt < 8:  # ab rows >= 1024 are zero; skip 9th tile
                    for ch in range(4):
                        nc.tensor.matmul(
                            pa4[ch],
                            lhsT=ab_t[kt],
                            rhs=xa_t[kt][:, ch * 512 : (ch + 1) * 512],
                            start=(kt == 0),
                            stop=(kt == 7),
                        )
                for c in bch:
                    c[kt]()
            for ch in range(4):
                cs = slice(ch * 512, (ch + 1) * 512)
                nc.vector.tensor_copy(ak_sb[0:R, cs], pa4[ch][0:R, :])
                nc.vector.tensor_copy(av_sb[0:R, cs], pa4[ch][2 * R : 3 * R, :])
            for c in bch:
                for step in c[NKT:]:
                    step()
        with tc.tile_pool(name="pPro2", bufs=3, space="PSUM") as pPro2:
            for ch in range(2):
                for step in emit_b_chunk(pPro2, 0, ch, pro_copy()):
                    step()
            for mt in range(3):
                for step in emit_c_chunk(pPro2, mt, pro_copy()):
                    step()

        # Attention: 8 single-head units with interleaved filler. Only the
        # PSUM pools live in this block; the norm-path SBUF/DRAM pools are
        # outer-scope so the block-close barrier doesn't serialize on the
        # final norm's DMA round-trips.
        pP = ctx.enter_context(tc.tile_pool(name="pP", bufs=8))
        pEv = ctx.enter_context(tc.tile_pool(name="pEv", bufs=3))
        pN = ctx.enter_context(tc.tile_pool(name="pN", bufs=3))
        pD = ctx.enter_context(tc.tile_pool(name="pD", bufs=3, space="DRAM"))
        with (
            tc.tile_pool(name="pS", bufs=4, space="PSUM") as pS,
            tc.tile_pool(name="pO", bufs=3, space="PSUM") as pO,
            tc.tile_pool(name="pX", bufs=1, space="PSUM") as pX,
        ):
            filler = []

            def alt_copy(i=[0]):
                i[0] += 1
                return nc.vector.tensor_copy if i[0] % 2 else nc.scalar.copy

            # queue order respects consumer deadlines: C[mt] before unit
            # (0,0)'s PV(mt); m3+m1(half0) before unit (0,2); m0(half1)
            # before unit (1,0); m1(half1) before unit (1,2)
            for mt in range(3, NTT):
                filler.extend(emit_c_chunk(pX, mt, alt_copy()))
            for m, ch in [(3, 0), (3, 1), (3, 2), (3, 3), (1, 0), (1, 1),
                          (0, 2), (0, 3), (1, 2), (1, 3)]:
                filler.extend(emit_b_chunk(pX, m, ch, alt_copy()))
            fill_pos = [0]

            def drain_filler(n):
                i = fill_pos[0]
                for _ in range(n):
                    if i >= len(filler):
                        break
                    filler[i]()
                    i += 1
                fill_pos[0] = i

            def emit_unit(half, h, fill_rate):
                hp = h // 2
                po = [
                    pO.tile([VW, 512], F32, tag="po", name=f"po_{half}_{h}_{c}")
                    for c in range(2)
                ]
                pts = {}

                def emit_pv(t):
                    pt = pts.pop(t)
                    for c in range(2):
                        nc.tensor.matmul(
                            po[c],
                            lhsT=v_sb[t][:, h * VW : (h + 1) * VW],
                            rhs=pt[:, c * 512 : (c + 1) * 512],
                            start=(t == 0),
                            stop=(t == NTT - 1),
                        )

                for tj in range(NTT):
                    pt = pP.tile([P, HF], BF, tag="pt", name=f"pt_{half}_{h}_{tj}")
                    for c in range(2):
                        s_ps = pS.tile([P, 512], F32, tag="s", name=f"ps_{half}_{h}_{tj}_{c}")
                        nc.tensor.matmul(
                            s_ps,
                            lhsT=kp_sb[h][:, tj * P : (tj + 1) * P],
                            rhs=qk_sb[hp][:, half * HF + c * 512 : half * HF + (c + 1) * 512],
                            start=True,
                            stop=True,
                        )
                        ptc = pt[:, c * 512 : (c + 1) * 512]
                        if c == 1 and tj in DVE_TJ:
                            nc.vector.tensor_scalar(
                                ptc.bitcast(I16), s_ps, EXP_A, EXP_B,
                                AluOpType.mult, AluOpType.add,
                            )
                        else:
                            nc.scalar.activation(
                                ptc, s_ps, mybir.ActivationFunctionType.Exp
                            )
                    pts[tj] = pt
                    drain_filler(fill_rate)
                    if tj > 1:
                        emit_pv(tj - 2)
                emit_pv(NTT - 2)
                emit_pv(NTT - 1)
                return po

            def emit_norm_rest(half, h, ev):
                hs = slice(half * HF, (half + 1) * HF)
                # SBUF->SBUF DMAs: spread the denominator row across 128
                # partitions, reciprocal on all lanes, reshape back, then
                # stride-0 broadcast -- one DMA hop fewer than via DRAM
                den128 = pN.tile([P, HF // P], BF, tag="d128", name=f"d128_{half}_{h}")
                nc.sync.dma_start(out=den128, in_=ev[HD:VW, :])
                rec = pN.tile([P, HF // P], BF, tag="rec", name=f"rec_{half}_{h}")
                with nc.allow_low_precision(
                    reason="softmax denom ~2048; bf16 recip adds ~0.4% row scale noise"
                ):
                    nc.vector.reciprocal(rec, den128)
                rw = pD.tile([1, HF], BF, tag="rw", name=f"rw_{half}_{h}")
                nc.sync.dma_start(
                    out=bass.AP(tensor=rw.tensor, offset=rw.offset,
                                ap=[[HF // P, P], [1, HF // P]]),
                    in_=rec,
                )
                rb = pN.tile([HD, HF], BF, tag="rb", name=f"rb_{half}_{h}")
                nc.sync.dma_start(
                    out=rb,
                    in_=bass.AP(tensor=rw.tensor, offset=rw.offset,
                                ap=[[0, HD], [1, HF]]),
                )
                nc.vector.tensor_mul(
                    oT_sb[h // 2][(h % 2) * HD : (h % 2) * HD + HD, hs],
                    ev[0:HD, :],
                    rb,
                )

            def emit_evac(half, h, po):
                ev = pEv.tile([VW, HF], BF, tag="ev", name=f"ev_{half}_{h}")
                for c in range(2):
                    nc.vector.tensor_copy(ev[:, c * 512 : (c + 1) * 512], po[c])
                return ev

            def emit_norm(half, h, po):
                emit_norm_rest(half, h, emit_evac(half, h, po))

            prev = None
            for half in range(2):
                for h in range(HPC):
                    rate = {(0, 0): 9, (0, 1): 6, (0, 2): 3}.get((half, h), 2)
                    po = emit_unit(half, h, rate)
                    if prev is not None:
                        emit_norm(*prev)
                        if (half, h) == (1, 0):
                            for mt in range(NTT // 2):
                                for ch in range(2):
                                    filler.extend(
                                        emit_outproj_chunk(pX, mt, ch, alt_copy())
                                    )
                    prev = (half, h, po)
            last_ev = (prev[0], prev[1], emit_evac(*prev))
            drain_filler(len(filler))

        # Epilogue: finish the last unit's norm outside the attention
        # pools (so the pool-close barrier doesn't serialize on its DMA
        # chain), then the half-1 out-projection in two 8-chunk waves --
        # each wave's kt2=0 matmuls depend only on the early half-1 norms
        # and bridge the remaining norm latency.
        with tc.tile_pool(name="pE", bufs=8, space="PSUM") as pE:
            eng = [nc.vector.tensor_copy, nc.scalar.copy]
            for wave in range(2):
                mts = range(NTT // 2 + wave * 4, NTT // 2 + wave * 4 + 4)
                chunks = [(mt, ch) for mt in mts for ch in range(2)]
                po2s = {
                    (mt, ch): pE.tile([P, 512], F32, tag="aux", name=f"po2e_{mt}_{ch}")
                    for (mt, ch) in chunks
                }
                for (mt, ch) in chunks:
                    nc.tensor.matmul(
                        po2s[(mt, ch)],
                        lhsT=oT_sb[0][:, mt * P : (mt + 1) * P],
                        rhs=wo_t[0][:, ch * 512 : (ch + 1) * 512],
                        start=True,
                        stop=False,
                    )
                if wave == 0:
                    # last norm's DMA chain emitted AFTER the kt2=0 pass so
                    # semaphore coarsening can't attach it to those matmuls;
                    # the kt2=1 pass below then carries the real dependency
                    emit_norm_rest(*last_ev)
                for (mt, ch) in chunks:
                    nc.tensor.matmul(
                        po2s[(mt, ch)],
                        lhsT=oT_sb[1][:, mt * P : (mt + 1) * P],
                        rhs=wo_t[1][:, ch * 512 : (ch + 1) * 512],
                        start=False,
                        stop=True,
                    )
                for j, (mt, ch) in enumerate(chunks):
                    if ch == 0:
                        ob_sb[mt] = pOut.tile([P, D], BF, tag="ob", name=f"ob_{mt}")
                    eng[j % 2](ob_sb[mt][:, ch * 512 : (ch + 1) * 512], po2s[(mt, ch)])
                    if ch == 1:
                        nc.sync.dma_start(out=out[mt, :, :], in_=ob_sb[mt])

    import bass_rust as _bass_rust

    _bass_rust.move_matmul_waits_to_ldweights(nc.m)
    _bass_rust.generate_event_semaphores(nc)
    return nc


def prepare_in_maps(inputs):
    q = np.asarray(inputs["query"], np.float32)
    ipw = np.asarray(inputs["in_proj_weight"], np.float32)
    ipb = np.asarray(inputs["in_proj_bias"], np.float32)
    out_w = np.asarray(inputs["out_w"], np.float32)
    k_a = np.asarray(inputs["k_a"], np.float32)
    k_b = np.asarray(inputs["k_b"], np.float32)
    v_a = np.asarray(inputs["v_a"], np.float32)
    v_b = np.asarray(inputs["v_b"], np.float32)
    qscale = 1.0 / math.sqrt(HD)
    sl = SCALE / R

    in_maps = []
    for c in range(NCORES):
        bb = c // 4
        s = (c % 4) * CD
        e = s + CD
        X = q[:, bb, :]

        xa = np.zeros((KPAD, T), np.float32)
        xa[:D] = X.T
        xa[D] = 1.0

        wqk = np.zeros((KPAD, 2 * CD), np.float32)
        wqk[:D, :CD] = ipw[s:e].T * qscale
        wqk[D, :CD] = ipb[s:e] * qscale
        wqk[:D, CD:] = ipw[D + s : D + e].T
        wqk[D, CD:] = ipb[D + s : D + e]

        wv = np.zeros((KPAD, HPC * VW), np.float32)
        for j in range(HPC):
            wv[:D, j * VW : j * VW + HD] = ipw[2 * D + s + j * HD : 2 * D + s + (j + 1) * HD].T
            wv[D, j * VW : j * VW + HD] = ipb[2 * D + s + j * HD : 2 * D + s + (j + 1) * HD]
            wv[D, j * VW + HD] = 1.0

        ab = np.zeros((KPAD, 3 * R), np.float32)
        ab[:D, :R] = k_a.T
        ab[:D, 2 * R :] = v_a.T

        kbm = k_b[:, s:e] * sl

        vbm = np.zeros((R, HPC * VW), np.float32)
        for j in range(HPC):
            vbm[:, j * VW : j * VW + HD] = v_b[:, s + j * HD : s + (j + 1) * HD] * sl

        wo = out_w[:, s:e].T

        in_maps.append(
            {
                "xa": xa.astype(BF16).reshape(NKT, P, T),
                "wqk": wqk.astype(BF16).reshape(NKT, P, 2 * CD),
                "wv": wv.astype(BF16).reshape(NKT, P, HPC * VW),
                "ab": ab.astype(BF16).reshape(NKT, P, 3 * R),
                "kbm": kbm.astype(BF16),
                "vbm": vbm.astype(BF16),
                "wo": wo.astype(BF16).reshape(2, P, D),
            }
        )
    return in_maps


def assemble_output(inputs, results):
    out_b = np.asarray(inputs["out_b"], np.float32)
    out = np.zeros((T, BSZ, D), np.float32)
    for c in range(NCORES):
        out[:, c // 4, :] += results[c]["out"].astype(np.float32).reshape(T, D)
    out += out_b[None, None, :]
    return out


def kernel(**inputs):
    nc = build_nc()
    in_maps = prepare_in_maps(inputs)
    res = run_bass_kernel_spmd(nc, in_maps, core_ids=list(range(NCORES)))
    return assemble_output(inputs, res.results)



# revision 19
# speedup vs baseline: 1.0912x; 1.0104x over previous
"""LoRA MultiheadAttention on 8 NeuronCores (Bass/Tile), v5.

Sharding: 32 (batch, head) attention slices -> 4 heads x 1 batch per core.
Cores 0-3 take batch 0, cores 4-7 batch 1; core c handles heads
(c%4)*4 .. (c%4)*4+3, i.e. a contiguous 256-wide slice of the head dims.

The PE is drain-bound on TRN2 (every matmul costs N fp32-PSUM-drain columns
at 1 col/cycle regardless of K/M), so the kernel keeps the PE instruction
stream dense end-to-end (all matmuls bf16; fp8 was tried and rejected:
e4m3's ~4% per-element noise does not average away in random GEMMs):

  prologue: A^T LoRA activations, all of Q^T/K^T, V row-tiles 0-2.
  attention: 8 single-head units x 16 tj iterations x 2 512-wide score
             chunks. 2 chunks/iter against 3 pS slots leaves a full exp of
             cross-iteration slack, so score matmuls rarely wait. The
             remaining 13 V row-tiles (paced ahead of their PV consumers)
             and the half-0 out-projection are drained into the PE stream
             as filler so exp waits never idle the PE; filler accumulates
             in a single spare PSUM bank.
  exp split: ACT (real exp) and DVE (one-op Schraudolph bf16:
             i16 = rint(s*128/ln2 + B) bitcast bf16, mean-zero calibrated,
             ~40% of chunks; softmax renormalizes, output err ~0.5%).
  norm:      po evacuated to bf16 SBUF immediately (frees PSUM banks);
             denominator row round-trips through DRAM reshaped to [128, 8]
             so the reciprocal uses 128 DVE lanes (0.13us vs 6.5us for a
             [64,1024] broadcast reciprocal); stride-0 DMA broadcast; one
             2x-mode bf16 multiply into oT_sb.
  epilogue:  half-1 out-projection, PSUM->SBUF copies alternating ACT/DVE.

b_v is folded into the V matmul ones-row bias; out_b added on host.
"""

import sys

sys.path.insert(0, "/opt/trn_rl_repo")

import math
from contextlib import ExitStack

import ml_dtypes
import numpy as np

import concourse.bass as bass
import concourse.tile as tile
from concourse import mybir
from concourse.alu_op_type import AluOpType
from concourse.bass_utils import run_bass_kernel_spmd

BF16 = ml_dtypes.bfloat16
F32 = mybir.dt.float32
BF = mybir.dt.bfloat16
I16 = mybir.dt.int16

T = 2048
D = 1024
H = 16
HD = 64
R = 16
BSZ = 2
SCALE = 16.0
NCORES = 8
HPC = 4  # heads per core
CD = HPC * HD  # 256 head dims per core
VW = HD + 1  # V block width per head (ones column appended)
KPAD = 1024  # 1024 X rows; biases folded into LoRA rows / Q evac
NKT = KPAD // 128
P = 128
NTT = T // P  # 16 row tiles
HF = T // 2  # 1024: ti processed in two halves

# Schraudolph-bf16 exp: i16 = rint(x * 128/ln2 + (127*128 - C)), bitcast bf16
EXP_A = 128.0 / math.log(2.0)
EXP_B = 127.0 * 128.0 - 7.3
# tj tiles whose c==1 exp chunk goes to DVE-Schraudolph (rest go to ACT)
DVE_TJ = frozenset(range(16)) - {5, 10, 15}


def build_nc():
    nc = bass.Bass()
    xa = nc.dram_tensor("xa", [NKT, P, T], BF, kind="ExternalInput")
    wqk = nc.dram_tensor("wqk", [NKT, P, 2 * CD], BF, kind="ExternalInput")
    wv = nc.dram_tensor("wv", [NKT, P, HPC * VW], BF, kind="ExternalInput")
    ab = nc.dram_tensor("ab", [NKT, P, 3 * R], BF, kind="ExternalInput")
    qb = nc.dram_tensor("qb", [2, P], mybir.dt.float32, kind="ExternalInput")
    kbm = nc.dram_tensor("kbm", [R + 1, CD], BF, kind="ExternalInput")
    vbm = nc.dram_tensor("vbm", [R + 1, HPC * VW], BF, kind="ExternalInput")
    wo = nc.dram_tensor("wo", [2, P, D], BF, kind="ExternalInput")
    out = nc.dram_tensor("out", [NTT, P, D], BF, kind="ExternalOutput")

    with tile.TileContext(nc) as tc, ExitStack() as ctx:
        singles = ctx.enter_context(tc.tile_pool(name="singles", bufs=1))

        xa_t = [singles.tile([P, T], BF, name=f"xa{i}", tag=f"xa{i}") for i in range(NKT)]
        wqk_t = [singles.tile([P, 2 * CD], BF, name=f"wqk{i}", tag=f"wqk{i}") for i in range(NKT)]
        wv_t = [singles.tile([P, HPC * VW], BF, name=f"wv{i}", tag=f"wv{i}") for i in range(NKT)]
        ab_t = [singles.tile([P, 3 * R], BF, name=f"ab{i}", tag=f"ab{i}") for i in range(NKT)]
        qb_t = singles.tile([P, 2], mybir.dt.float32, tag="qb")
        kb_t = singles.tile([P, CD], BF, tag="kb")
        vb_t = singles.tile([P, HPC * VW], BF, tag="vb")
        nc.vector.memset(kb_t, 0.0)
        nc.vector.memset(vb_t, 0.0)
        wo_t = [singles.tile([P, D], BF, name=f"wo{i}", tag=f"wo{i}") for i in range(2)]
        # load order matches consumption: A needs ab+xa, then B needs wqk.
        # First k-tile split across rings; dispatch alternates the two
        # HW-DGE queues (Sync + ACT) to halve descriptor-issue serialization
        eng_i = [0]

        def ld(dst, src):
            eng_i[0] += 1
            (nc.sync if eng_i[0] % 2 else nc.scalar).dma_start(out=dst, in_=src)

        for p4 in range(4):
            ps = slice(p4 * 32, (p4 + 1) * 32)
            ld(ab_t[0][ps, :], ab[0, ps, :])
        for p2 in range(2):
            ps = slice(p2 * 64, (p2 + 1) * 64)
            ld(xa_t[0][ps, 0:512], xa[0, ps, 0:512])
        for p2 in range(2):
            ps = slice(p2 * 64, (p2 + 1) * 64)
            ld(wqk_t[0][ps, :], wqk[0, ps, :])
        # qb arrives transposed [2,128] -> [128,2] (tiny, descriptor-heavy;
        # dispatched early, first consumed ~20us in)
        ld(qb_t, qb[:, :].rearrange("a b -> b a"))
        for c4 in range(1, 4):
            cs = slice(c4 * 512, (c4 + 1) * 512)
            ld(xa_t[0][:, cs], xa[0, :, cs])
        for i in range(1, NKT):
            ld(ab_t[i], ab[i, :, :])
            ld(xa_t[i], xa[i, :, :])
            ld(wqk_t[i], wqk[i, :, :])
        ld(kb_t[0:R, :], kbm[0:R, :])
        ld(kb_t[32:33, :], kbm[R : R + 1, :])
        ld(vb_t[0:R, :], vbm[0:R, :])
        ld(vb_t[32:33, :], vbm[R : R + 1, :])
        for i in range(NKT):
            ld(wv_t[i], wv[i, :, :])
        for i in range(2):
            ld(wo_t[i], wo[i, :, :])

        # Q^T tiles (heads 0-1 / 2-3); K^T stored per head zero-padded to
        # 128 contraction rows so every attention matmul runs in the PE's
        # (128,128) tiling mode -- mode switches drain the whole array.
        qk_sb = [singles.tile([P, T], BF, name=f"qk{i}", tag=f"qk{i}") for i in range(2)]
        kp_sb = [singles.tile([P, T], BF, name=f"kp{i}", tag=f"kp{i}") for i in range(HPC)]
        ak_sb = singles.tile([P, T], BF, tag="ak")
        av_sb = singles.tile([P, T], BF, tag="av")
        for h8, t8 in enumerate(kp_sb):
            pad = slice(HD, P) if h8 % 2 == 0 else slice(0, HD)
            (nc.vector if h8 < 2 else nc.gpsimd).memset(t8[pad, :], 0.0)
        nc.vector.memset(ak_sb, 0.0)
        nc.vector.memset(ak_sb[32:33, :], 1.0)
        nc.gpsimd.memset(av_sb, 0.0)
        nc.vector.memset(av_sb[32:33, :], 1.0)
        v_sb = [singles.tile([P, HPC * VW], BF, name=f"v{i}", tag=f"v{i}") for i in range(NTT)]
        oT_sb = [singles.tile([P, T], BF, name=f"oT{i}", tag=f"oT{i}") for i in range(2)]
        pOut = ctx.enter_context(tc.tile_pool(name="pOut", bufs=3))
        ob_sb = {}

        def emit_b_chunk(pool, m, ch, copy_engine):
            cs = slice(ch * 512, (ch + 1) * 512)
            pq = pool.tile([P, 512], F32, tag="aux", name=f"pq_{m}_{ch}")
            steps = []
            for kt in range(NKT):
                steps.append(
                    lambda kt=kt, pq=pq: nc.tensor.matmul(
                        pq,
                        lhsT=wqk_t[kt][:, m * P : (m + 1) * P],
                        rhs=xa_t[kt][:, cs],
                        start=(kt == 0),
                        stop=(kt == NKT - 1 and m < 2),
                    )
                )
            if m >= 2:
                steps.append(
                    lambda pq=pq: nc.tensor.matmul(
                        pq,
                        lhsT=kb_t[:, (m - 2) * P : (m - 1) * P],
                        rhs=ak_sb[:, cs],
                        start=False,
                        stop=True,
                    )
                )
                h0 = 2 * (m - 2)
                steps.append(
                    lambda pq=pq, h0=h0: copy_engine(
                        kp_sb[h0][0:HD, cs], pq[0:HD, :]
                    )
                )
                steps.append(
                    lambda pq=pq, h0=h0: copy_engine(
                        kp_sb[h0 + 1][HD:P, cs], pq[HD:P, :]
                    )
                )
            else:
                steps.append(
                    lambda pq=pq: nc.vector.tensor_scalar(
                        qk_sb[m][:, cs], pq, qb_t[:, m : m + 1], None,
                        AluOpType.add,
                    )
                )
            return steps

        def emit_c_chunk(pool, mt, copy_engine):
            ms = slice(mt * P, (mt + 1) * P)
            pv = pool.tile([P, HPC * VW], F32, tag="aux", name=f"pv_{mt}")
            steps = []
            for kt in range(NKT):
                steps.append(
                    lambda kt=kt, pv=pv: nc.tensor.matmul(
                        pv,
                        lhsT=xa_t[kt][:, ms],
                        rhs=wv_t[kt],
                        start=(kt == 0),
                        stop=False,
                    )
                )
            steps.append(
                lambda pv=pv: nc.tensor.matmul(
                    pv, lhsT=av_sb[:, ms], rhs=vb_t, start=False, stop=True
                )
            )
            steps.append(lambda pv=pv: copy_engine(v_sb[mt], pv))
            return steps

        def emit_outproj_chunk(pool, mt, ch, copy_engine):
            ms = slice(mt * P, (mt + 1) * P)
            cs = slice(ch * 512, (ch + 1) * 512)
            po2 = pool.tile([P, 512], F32, tag="aux", name=f"po2_{mt}_{ch}")
            steps = []
            if ch == 0:
                def mkob(mt=mt):
                    ob_sb[mt] = pOut.tile([P, D], BF, tag="ob", name=f"ob_{mt}")
                steps.append(mkob)
            for kt2 in range(2):
                steps.append(
                    lambda kt2=kt2, po2=po2: nc.tensor.matmul(
                        po2,
                        lhsT=oT_sb[kt2][:, ms],
                        rhs=wo_t[kt2][:, cs],
                        start=(kt2 == 0),
                        stop=(kt2 == 1),
                    )
                )

            def fin(po2=po2, mt=mt, ch=ch):
                copy_engine(ob_sb[mt][:, cs], po2)
                if ch == 1:
                    nc.sync.dma_start(out=out[mt, :, :], in_=ob_sb[mt])

            steps.append(fin)
            return steps

        def pro_copy(i=[0]):
            i[0] += 1
            return nc.vector.tensor_copy if i[0] % 2 else nc.scalar.copy

        # Phase A + K^T(m2) prologue, kt-major: each k-tile of every
        # accumulation chain is consumed as soon as its DMA lands, so the
        # PE tracks the input load instead of waiting for it.
        with tc.tile_pool(name="pA", bufs=2, space="PSUM") as pA, \
             tc.tile_pool(name="pPro", bufs=4, space="PSUM") as pPro:
            # chunk pairs (2i, 2i+1) at out partitions 0:48 / 64:112 run
            # concurrently on the PE's column groups
            pa2 = [pA.tile([P, 512], F32, tag="pa", name=f"pa{i}") for i in range(2)]
            bch = [emit_b_chunk(pPro, 2, ch, pro_copy()) for ch in range(4)]
            for kt in range(NKT):
                for i in range(2):
                    nc.tensor.matmul(
                        pa2[i][0:48, :],
                        lhsT=ab_t[kt],
                        rhs=xa_t[kt][:, (2 * i) * 512 : (2 * i + 1) * 512],
                        start=(kt == 0),
                        stop=(kt == NKT - 1),
                    )
                    nc.tensor.matmul(
                        pa2[i][64:112, :],
                        lhsT=ab_t[kt],
                        rhs=xa_t[kt][:, (2 * i + 1) * 512 : (2 * i + 2) * 512],
                        start=(kt == 0),
                        stop=(kt == NKT - 1),
                    )
                for c in bch:
                    c[kt]()
            for i in range(2):
                for (r0, dst) in ((0, ak_sb), (2 * R, av_sb)):
                    cs0 = slice((2 * i) * 512, (2 * i + 1) * 512)
                    cs1 = slice((2 * i + 1) * 512, (2 * i + 2) * 512)
                    nc.vector.tensor_copy(dst[0:R, cs0], pa2[i][r0 : r0 + R, :])
                    nc.vector.tensor_copy(dst[0:R, cs1], pa2[i][64 + r0 : 64 + r0 + R, :])
            for c in bch:
                for step in c[NKT:]:
                    step()
        with tc.tile_pool(name="pPro2", bufs=3, space="PSUM") as pPro2:
            for ch in range(2):
                for step in emit_b_chunk(pPro2, 0, ch, pro_copy()):
                    step()
            for mt in range(3):
                for step in emit_c_chunk(pPro2, mt, pro_copy()):
                    step()

        # Attention: 8 single-head units with interleaved filler. Only the
        # PSUM pools live in this block; the norm-path SBUF/DRAM pools are
        # outer-scope so the block-close barrier doesn't serialize on the
        # final norm's DMA round-trips.
        pP = ctx.enter_context(tc.tile_pool(name="pP", bufs=8))
        pEv = ctx.enter_context(tc.tile_pool(name="pEv", bufs=3))
        pN = ctx.enter_context(tc.tile_pool(name="pN", bufs=3))
        pD = ctx.enter_context(tc.tile_pool(name="pD", bufs=3, space="DRAM"))
        with (
            tc.tile_pool(name="pS", bufs=4, space="PSUM") as pS,
            tc.tile_pool(name="pO", bufs=3, space="PSUM") as pO,
            tc.tile_pool(name="pX", bufs=1, space="PSUM") as pX,
        ):
            filler = []

            def alt_copy(i=[0]):
                i[0] += 1
                return nc.vector.tensor_copy if i[0] % 2 else nc.scalar.copy

            # queue order respects consumer deadlines: C[mt] before unit
            # (0,0)'s PV(mt); m3+m1(half0) before unit (0,2); m0(half1)
            # before unit (1,0); m1(half1) before unit (1,2)
            for mt in range(3, NTT):
                filler.extend(emit_c_chunk(pX, mt, alt_copy()))
            for m, ch in [(3, 0), (3, 1), (3, 2), (3, 3), (1, 0), (1, 1),
                          (0, 2), (0, 3), (1, 2), (1, 3)]:
                filler.extend(emit_b_chunk(pX, m, ch, alt_copy()))
            fill_pos = [0]

            def drain_filler(n):
                i = fill_pos[0]
                for _ in range(n):
                    if i >= len(filler):
                        break
                    filler[i]()
                    i += 1
                fill_pos[0] = i

            def emit_unit(half, h, fill_rate):
                hp = h // 2
                po = [
                    pO.tile([VW, 512], F32, tag="po", name=f"po_{half}_{h}_{c}")
                    for c in range(2)
                ]
                pts = {}

                def emit_pv(t):
                    pt = pts.pop(t)
                    for c in range(2):
                        nc.tensor.matmul(
                            po[c],
                            lhsT=v_sb[t][:, h * VW : (h + 1) * VW],
                            rhs=pt[:, c * 512 : (c + 1) * 512],
                            start=(t == 0),
                            stop=(t == NTT - 1),
                        )

                for tj in range(NTT):
                    pt = pP.tile([P, HF], BF, tag="pt", name=f"pt_{half}_{h}_{tj}")
                    for c in range(2):
                        s_ps = pS.tile([P, 512], F32, tag="s", name=f"ps_{half}_{h}_{tj}_{c}")
                        nc.tensor.matmul(
                            s_ps,
                            lhsT=kp_sb[h][:, tj * P : (tj + 1) * P],
                            rhs=qk_sb[hp][:, half * HF + c * 512 : half * HF + (c + 1) * 512],
                            start=True,
                            stop=True,
                        )
                        ptc = pt[:, c * 512 : (c + 1) * 512]
                        if c == 1 and tj in DVE_TJ:
                            nc.vector.tensor_scalar(
                                ptc.bitcast(I16), s_ps, EXP_A, EXP_B,
                                AluOpType.mult, AluOpType.add,
                            )
                        else:
                            nc.scalar.activation(
                                ptc, s_ps, mybir.ActivationFunctionType.Exp
                            )
                    pts[tj] = pt
                    drain_filler(fill_rate)
                    if tj > 1:
                        emit_pv(tj - 2)
                emit_pv(NTT - 2)
                emit_pv(NTT - 1)
                return po

            def emit_norm_rest(half, h, ev):
                hs = slice(half * HF, (half + 1) * HF)
                # SBUF->SBUF DMAs: spread the denominator row across 128
                # partitions, reciprocal on all lanes, reshape back, then
                # stride-0 broadcast -- one DMA hop fewer than via DRAM
                den128 = pN.tile([P, HF // P], BF, tag="d128", name=f"d128_{half}_{h}")
                nc.sync.dma_start(out=den128, in_=ev[HD:VW, :])
                rec = pN.tile([P, HF // P], BF, tag="rec", name=f"rec_{half}_{h}")
                with nc.allow_low_precision(
                    reason="softmax denom ~2048; bf16 recip adds ~0.4% row scale noise"
                ):
                    nc.vector.reciprocal(rec, den128)
                rw = pD.tile([1, HF], BF, tag="rw", name=f"rw_{half}_{h}")
                nc.sync.dma_start(
                    out=bass.AP(tensor=rw.tensor, offset=rw.offset,
                                ap=[[HF // P, P], [1, HF // P]]),
                    in_=rec,
                )
                rb = pN.tile([HD, HF], BF, tag="rb", name=f"rb_{half}_{h}")
                nc.sync.dma_start(
                    out=rb,
                    in_=bass.AP(tensor=rw.tensor, offset=rw.offset,
                                ap=[[0, HD], [1, HF]]),
                )
                nc.vector.tensor_mul(
                    oT_sb[h // 2][(h % 2) * HD : (h % 2) * HD + HD, hs],
                    ev[0:HD, :],
                    rb,
                )

            def emit_evac(half, h, po):
                ev = pEv.tile([VW, HF], BF, tag="ev", name=f"ev_{half}_{h}")
                for c in range(2):
                    nc.vector.tensor_copy(ev[:, c * 512 : (c + 1) * 512], po[c])
                return ev

            def emit_norm(half, h, po):
                emit_norm_rest(half, h, emit_evac(half, h, po))

            prev = None
            for half in range(2):
                for h in range(HPC):
                    rate = {(0, 0): 9, (0, 1): 6, (0, 2): 3}.get((half, h), 2)
                    po = emit_unit(half, h, rate)
                    if prev is not None:
                        emit_norm(*prev)
                        if (half, h) == (1, 0):
                            for mt in range(NTT // 2):
                                for ch in range(2):
                                    filler.extend(
                                        emit_outproj_chunk(pX, mt, ch, alt_copy())
                                    )
                    prev = (half, h, po)
            last_ev = (prev[0], prev[1], emit_evac(*prev))
            drain_filler(len(filler))

        # Epilogue: finish the last unit's norm outside the attention
        # pools (so the pool-close barrier doesn't serialize on its DMA
        # chain), then the half-1 out-projection in two 8-chunk waves --
        # each wave's kt2=0 matmuls depend only on the early half-1 norms
        # and bridge the remaining norm latency.
        with tc.tile_pool(name="pE", bufs=8, space="PSUM") as pE:
            eng = [nc.vector.tensor_copy, nc.scalar.copy]
            for wave in range(2):
                mts = range(NTT // 2 + wave * 4, NTT // 2 + wave * 4 + 4)
                chunks = [(mt, ch) for mt in mts for ch in range(2)]
                po2s = {
                    (mt, ch): pE.tile([P, 512], F32, tag="aux", name=f"po2e_{mt}_{ch}")
                    for (mt, ch) in chunks
                }
                for (mt, ch) in chunks:
                    nc.tensor.matmul(
                        po2s[(mt, ch)],
                        lhsT=oT_sb[0][:, mt * P : (mt + 1) * P],
                        rhs=wo_t[0][:, ch * 512 : (ch + 1) * 512],
                        start=True,
                        stop=False,
                    )
                if wave == 0:
                    # last norm's DMA chain emitted AFTER the kt2=0 pass so
                    # semaphore coarsening can't attach it to those matmuls;
                    # the kt2=1 pass below then carries the real dependency
                    emit_norm_rest(*last_ev)
                for (mt, ch) in chunks:
                    nc.tensor.matmul(
                        po2s[(mt, ch)],
                        lhsT=oT_sb[1][:, mt * P : (mt + 1) * P],
                        rhs=wo_t[1][:, ch * 512 : (ch + 1) * 512],
                        start=False,
                        stop=True,
                    )
                for j, (mt, ch) in enumerate(chunks):
                    if ch == 0:
                        ob_sb[mt] = pOut.tile([P, D], BF, tag="ob", name=f"ob_{mt}")
                    eng[j % 2](ob_sb[mt][:, ch * 512 : (ch + 1) * 512], po2s[(mt, ch)])
                    if ch == 1:
                        nc.sync.dma_start(out=out[mt, :, :], in_=ob_sb[mt])

    import bass_rust as _bass_rust

    _bass_rust.move_matmul_waits_to_ldweights(nc.m)
    _bass_rust.generate_event_semaphores(nc)
    return nc


def prepare_in_maps(inputs):
    q = np.asarray(inputs["query"], np.float32)
    ipw = np.asarray(inputs["in_proj_weight"], np.float32)
    ipb = np.asarray(inputs["in_proj_bias"], np.float32)
    out_w = np.asarray(inputs["out_w"], np.float32)
    k_a = np.asarray(inputs["k_a"], np.float32)
    k_b = np.asarray(inputs["k_b"], np.float32)
    v_a = np.asarray(inputs["v_a"], np.float32)
    v_b = np.asarray(inputs["v_b"], np.float32)
    qscale = 1.0 / math.sqrt(HD)
    sl = SCALE / R

    in_maps = []
    for c in range(NCORES):
        bb = c // 4
        s = (c % 4) * CD
        e = s + CD
        X = q[:, bb, :]

        xa = X.T.copy()

        wqk = np.zeros((D, 2 * CD), np.float32)
        wqk[:, :CD] = ipw[s:e].T * qscale
        wqk[:, CD:] = ipw[D + s : D + e].T

        qbm = (ipb[s:e] * qscale).reshape(2, P)

        wv = np.zeros((D, HPC * VW), np.float32)
        for j in range(HPC):
            wv[:, j * VW : j * VW + HD] = ipw[2 * D + s + j * HD : 2 * D + s + (j + 1) * HD].T

        ab = np.zeros((D, 3 * R), np.float32)
        ab[:, :R] = k_a.T
        ab[:, 2 * R :] = v_a.T

        kbm = np.zeros((R + 1, CD), np.float32)
        kbm[:R] = k_b[:, s:e] * sl
        kbm[R] = ipb[D + s : D + e]

        vbm = np.zeros((R + 1, HPC * VW), np.float32)
        for j in range(HPC):
            vbm[:R, j * VW : j * VW + HD] = v_b[:, s + j * HD : s + (j + 1) * HD] * sl
            vbm[R, j * VW : j * VW + HD] = ipb[2 * D + s + j * HD : 2 * D + s + (j + 1) * HD]
            vbm[R, j * VW + HD] = 1.0

        wo = out_w[:, s:e].T

        in_maps.append(
            {
                "xa": xa.astype(BF16).reshape(NKT, P, T),
                "wqk": wqk.astype(BF16).reshape(NKT, P, 2 * CD),
                "wv": wv.astype(BF16).reshape(NKT, P, HPC * VW),
                "ab": ab.astype(BF16).reshape(NKT, P, 3 * R),
                "qb": qbm.astype(np.float32),
                "kbm": kbm.astype(BF16),
                "vbm": vbm.astype(BF16),
                "wo": wo.astype(BF16).reshape(2, P, D),
            }
        )
    return in_maps


def assemble_output(inputs, results):
    out_b = np.asarray(inputs["out_b"], np.float32)
    out = np.zeros((T, BSZ, D), np.float32)
    for c in range(NCORES):
        out[:, c // 4, :] += results[c]["out"].astype(np.float32).reshape(T, D)
    out += out_b[None, None, :]
    return out


def kernel(**inputs):
    nc = build_nc()
    in_maps = prepare_in_maps(inputs)
    res = run_bass_kernel_spmd(nc, in_maps, core_ids=list(range(NCORES)))
    return assemble_output(inputs, res.results)

